# revision 1
# baseline (speedup 1.0000x reference)
"""AlphaFold-style gated MSA attention on 8 Trainium2 NeuronCores.

Batch-sharded (128 batches -> 16 per core). Full inputs in, full output out.

Math per batch b (reference):
  q = (q_data @ Wq) * hk^-0.5          [Q, H, 32]
  k = m_data @ Wk ; v = m_data @ Wv    [K, H, 32]
  S[h] = q_h k_h^T + bias[b] + nb[h]   [H, Q, K]
  w = softmax(S, axis=-1)
  wa = w @ v                            [Q, H, 32]
  gate = sigmoid(q_data @ Wg + gb)
  out = (wa * gate).reshape(Q, 256) @ Wo + o_bias

Device-side formulation (per core, layouts chosen so no transposes are
needed on-device):
  S^T[k, q] computed head-by-head from k^T/q^T projections (feature dim on
  partitions).  softmax is done unnormalized with the bias adds replaced by
  multiplies of host-precomputed exp(bias)^T ("eb") and exp(nb)^T ("en"):
      w^T = exp(S^T) * en_h * eb          (bf16)
  The V-matmul uses lhsT = [v_h | 2.0] so PSUM row 32 accumulates 2*sum_k w,
  giving the softmax denominators for free.  Normalization and gating fuse:
      ga^T = wa^T * (1 + tanh(x/2 + gb/2)) * recip(2*sum) = wa^T*sigmoid/sum
  with the per-head recip broadcast across 32 partitions by a tiny indicator
  matmul.  Output projection back to [q, 256] with o_bias added during PSUM
  evacuation.
"""

import os
import sys

sys.path.insert(0, "/opt/trn_rl_repo")

import numpy as np
import ml_dtypes
from contextlib import ExitStack

import concourse.bass as bass  # noqa: F401  (engine types)
import concourse.bacc as bacc
import concourse.mybir as mybir
import concourse.tile as tile

BF16 = ml_dtypes.bfloat16

NUM_CORES = 8
B, Q, K, A = 128, 384, 384, 256
H, HD = 8, 32  # heads, head dim
OUT = 256
BPC = B // NUM_CORES  # batches per core


PAIR_MUL = __import__("os").environ.get("PAIR_MUL", "0") == "1"
_pm = __import__("os").environ.get("PREMUL_HEADS", "0,2,4,6")
PREMUL_HEADS = tuple(int(x) for x in _pm.split(",") if x != "")
GPS_HEADS = tuple(int(x) for x in __import__("os").environ.get("GPS_HEADS", "9").split(","))


def _build_body(ctx, tc, io, bpc):
    nc = tc.nc
    f32, bf = mybir.dt.float32, mybir.dt.bfloat16
    Exp = mybir.ActivationFunctionType.Exp
    Tanh = mybir.ActivationFunctionType.Tanh
    MUL, ADD = mybir.AluOpType.mult, mybir.AluOpType.add

    const = ctx.enter_context(tc.tile_pool(name="const", bufs=1))
    lp = ctx.enter_context(tc.tile_pool(name="loads", bufs=int(__import__("os").environ.get("LP_BUFS", "5"))))
    pp = ctx.enter_context(tc.tile_pool(name="proj", bufs=int(__import__("os").environ.get("PP_BUFS", "3"))))
    wp = ctx.enter_context(tc.tile_pool(name="work", bufs=int(__import__("os").environ.get("WP_BUFS", "4"))))
    wap = ctx.enter_context(tc.tile_pool(name="wa", bufs=8))
    gp = ctx.enter_context(tc.tile_pool(name="gating", bufs=int(__import__("os").environ.get("GP_BUFS", "3"))))
    outp = ctx.enter_context(tc.tile_pool(name="outp", bufs=3))
    # PSUM: 2 x 3 banks (S^T) + 2 x 1 bank (everything else) = 8 banks.
    Sp = ctx.enter_context(tc.tile_pool(name="psum_S", bufs=2, space="PSUM"))
    sp = ctx.enter_context(tc.tile_pool(name="psum_sm", bufs=2, space="PSUM"))

    # ---- resident constants ----
    en_sb = const.tile([128, H, 3, Q], bf, tag="en")
    nc.sync.dma_start(en_sb[:], io["enT"])
    w_sb = {}
    for name in ("wq", "wk", "wv", "wg", "wo"):
        w_sb[name] = const.tile([128, 2, 256], bf, tag=name, name=name)
        nc.sync.dma_start(w_sb[name][:], io[name])
    # o_bias as a [1, 256] row plus a [1, 128] ones row for the rank-1
    # PSUM-accumulate trick (bf16 to match the other matmul operands)
    obias_row = const.tile([1, OUT], bf, tag="obias_row")
    nc.sync.dma_start(obias_row[:], io["obias_bf"])
    ones_row = const.tile([1, 128], bf, tag="ones_row")
    nc.sync.dma_start(ones_row[:], io["ind"][127:128, 0:128])
    gbh_sb = const.tile([128, 2], f32, tag="gbh")
    nc.sync.dma_start(gbh_sb[:], io["gbh"])
    if os.environ.get("ACT_WARM", "0") == "1":
        # dummy activation right after the first tiny const DMA: pulls the
        # ~2.7us ACT table load off the critical path (exp_and_others holds
        # both Exp and Tanh, so no further loads fire later)
        warm = const.tile([128, 2], f32, tag="warm")
        nc.scalar.activation(warm[:], gbh_sb[:], Exp)
    ind_sb = const.tile([128, 256], bf, tag="ind")
    nc.sync.dma_start(ind_sb[:], io["ind"])
    if os.environ.get("RBC_F32", "0") == "1":
        ind_f_sb = const.tile([8, 256], f32, tag="ind_f")
        nc.sync.dma_start(ind_f_sb[:], io["ind_f"])
    else:
        ind_f_sb = None

    def emit_tail(b, sums_bf, waA, gt):
        import contextlib
        _hp = (
            tc.high_priority()
            if os.environ.get("TAIL_HIPRI", "0") == "1"
            else contextlib.nullcontext()
        )
        with _hp:
            return _emit_tail_inner(b, sums_bf, waA, gt)

    def _emit_tail_inner(b, sums_bf, waA, gt):
        # ---- normalization + gating + output projection (batch tail) ----
        if os.environ.get("WA_F32", "0") == "1":
            sums_f = sums_bf  # already fp32
        else:
            sums_f = gp.tile([8, Q], f32, tag="sums_f", name=f"sums_f_{b}")
            _sf = os.environ.get("SF_ENG", "dve")
            if _sf == "gps":
                nc.gpsimd.tensor_copy(sums_f[:], sums_bf[:])
            elif _sf == "act":
                nc.scalar.copy(sums_f[:], sums_bf[:])
            else:
                nc.vector.tensor_copy(sums_f[:], sums_bf[:])
        rec = gp.tile([8, Q], f32, tag="rec", name=f"rec_{b}")
        nc.vector.reciprocal_approx_fast(rec[:], sums_f[:])
        if os.environ.get("RBC_F32", "0") == "1":
            recb = rec  # fp32 path: indicator matmul runs in fp32, no cast
            ind_mm = ind_f_sb
        else:
            recb = gp.tile([8, Q], bf, tag="recb", name=f"recb_{b}")
            (nc.gpsimd.tensor_copy if os.environ.get("RECB_GPS", "0") == "1" else nc.vector.tensor_copy)(recb[:], rec[:])
            ind_mm = ind_sb
        ga_tiles = []
        for j in range(2):
            _pool_r = Sp if os.environ.get("PSR_S", "0") == "1" else sp
            _tag_r = "S" if os.environ.get("PSR_S", "0") == "1" else "sm"
            psR = _pool_r.tile(
                [128, 512], f32, tag=_tag_r, name=f"psR{j}_{b}"
            )
            nc.tensor.matmul(
                psR[:, :Q],
                ind_mm[0:8, 128 * j : 128 * (j + 1)],
                recb[:],
                start=True,
                stop=True,
            )
            g2 = gp.tile([128, Q], bf, tag="g2", name=f"g2{j}_{b}")
            if os.environ.get("RBC_EVAC", "0") == "1":
                # ACT evacuates rbc so the STT runs in bf16 2x mode
                rbc_sb = gp.tile([128, Q], bf, tag="rbc_sb", name=f"rbc{j}_{b}")
                nc.scalar.copy(rbc_sb[:], psR[:, :Q])
                nc.vector.scalar_tensor_tensor(
                    g2[:], gt[:, j, :], 1.0, rbc_sb[:], op0=ADD, op1=MUL
                )
            else:
                nc.vector.scalar_tensor_tensor(
                    g2[:], gt[:, j, :], 1.0, psR[:, :Q], op0=ADD, op1=MUL
                )
            ga = gp.tile([128, Q], bf, tag="ga", name=f"ga{j}_{b}")
            (nc.gpsimd if os.environ.get("GA_GPS", "0") == "1" else nc.vector).tensor_tensor(
                ga[:], waA[j][:], g2[:], op=MUL
            )
            ga_tiles.append(ga)
        ob = outp.tile([128, 3, OUT], f32, tag="ob", name=f"ob_{b}")
        for qc in range(3):
            _pool_o = Sp if os.environ.get("PSO_S", "0") == "1" else sp
            _tag_o = "S" if os.environ.get("PSO_S", "0") == "1" else "sm"
            psO = _pool_o.tile(
                [128, 512], f32, tag=_tag_o, name=f"psO{qc}_{b}"
            )
            for j in range(2):
                nc.tensor.matmul(
                    psO[:, :OUT],
                    ga_tiles[j][:, 128 * qc : 128 * (qc + 1)],
                    w_sb["wo"][:, j, :],
                    start=(j == 0),
                    stop=False,
                )
            # rank-1 accumulate of o_bias: ones_col.T @ obias_row
            nc.tensor.matmul(
                psO[:, :OUT],
                ones_row[:],
                obias_row[:],
                start=False,
                stop=True,
            )
            (nc.scalar.copy if os.environ.get("OUT_ACT", "1") == "1" else nc.vector.tensor_copy)(ob[:, qc, :], psO[:, :OUT])
        if os.environ.get("OUT_SPLIT", "0") == "1":
            for qc in range(3):
                nc.sync.dma_start(
                    io["out"][b, 128 * qc : 128 * (qc + 1), :], ob[:, qc, :]
                )
        else:
            (nc.scalar if os.environ.get("ODMA_ACT", "0") == "1" else nc.sync).dma_start(
                io["out"][b].rearrange("(c p) o -> p c o", p=128), ob[:]
            )

    def emit_loads_proj(b):
        # ---- loads ----
        qd = lp.tile([128, 2, Q], bf, tag="qd", name=f"qd_{b}")
        nc.sync.dma_start(qd[:], io["inT"][b, :, 0:2, :])
        md = lp.tile([128, 2, Q], bf, tag="md", name=f"md_{b}")
        nc.sync.dma_start(md[:], io["inT"][b, :, 2:4, :])
        eb = lp.tile([128, 3, Q], bf, tag="eb", name=f"eb_{b}")
        nc.sync.dma_start(eb[:], io["inT"][b, :, 4:7, :])

        # ---- projections ----
        qT = pp.tile([128, 2, Q], bf, tag="qT", name=f"qT_{b}")  # [hc, j, q]
        kT = pp.tile([128, 2, Q], bf, tag="kT", name=f"kT_{b}")  # [hc, j, k]
        gt = pp.tile([128, 2, Q], bf, tag="gt", name=f"gt_{b}")
        # [k, kc, h*33+c | 2.0]; padded to 320 so every head has a 64-wide
        # lhsT window (M=64 writes initialized junk to PSUM rows 32-63,
        # letting the pair evacuation be one full-width copy).
        vv = pp.tile([128, 3, 320], bf, tag="vv", name=f"vv_{b}")
        nc.gpsimd.memset(vv[:], 2.0)

        def emit_gproj(j):
            ps = sp.tile([128, 512], f32, tag="sm", name=f"psg{j}_{b}")
            for a in range(2):
                nc.tensor.matmul(
                    ps[:, :Q],
                    w_sb["wg"][:, a, 128 * j : 128 * (j + 1)],
                    qd[:, a, :],
                    start=(a == 0),
                    stop=(a == 1),
                )
            nc.scalar.activation(
                gt[:, j, :], ps[:, :Q], Tanh, bias=gbh_sb[:, j : j + 1], scale=0.5
            )

        def emit_vproj():
            for kc in range(3):
                ps = sp.tile([128, 512], f32, tag="sm", name=f"psv{kc}_{b}")
                for a in range(2):
                    nc.tensor.matmul(
                        ps[:, :256],
                        md[:, a, 128 * kc : 128 * (kc + 1)],
                        w_sb["wv"][:, a, :],
                        start=(a == 0),
                        stop=(a == 1),
                    )
                _ev = (
                    nc.scalar.copy
                    if os.environ.get("VV_ACT", "1") == "1"
                    else nc.vector.tensor_copy
                )
                _ev(
                    vv[:, kc, 0:264].rearrange("p (h c) -> p h c", c=33)[:, :, 0:32],
                    ps[:, :256].rearrange("p (h c) -> p h c", c=32),
                )

        for j in range(2):
            ps = sp.tile([128, 512], f32, tag="sm", name=f"psq{j}_{b}")
            for a in range(2):
                nc.tensor.matmul(
                    ps[:, :Q],
                    w_sb["wq"][:, a, 128 * j : 128 * (j + 1)],
                    qd[:, a, :],
                    start=(a == 0),
                    stop=(a == 1),
                )
            (nc.scalar.copy if os.environ.get("QK_ACT", "0") in ("1", "q") else nc.vector.tensor_copy)(qT[:, j, :], ps[:, :Q])
            ps = sp.tile([128, 512], f32, tag="sm", name=f"psk{j}_{b}")
            for a in range(2):
                nc.tensor.matmul(
                    ps[:, :Q],
                    w_sb["wk"][:, a, 128 * j : 128 * (j + 1)],
                    md[:, a, :],
                    start=(a == 0),
                    stop=(a == 1),
                )
            (nc.scalar.copy if os.environ.get("QK_ACT", "0") == "1" else nc.vector.tensor_copy)(kT[:, j, :], ps[:, :Q])
            if os.environ.get("PROJ_ORDER", "a") == "a":
                emit_gproj(j)
        if os.environ.get("PROJ_ORDER", "a") == "b":
            # v first (needed by the first V-matmul early in the heads);
            # gate last (not needed until the batch tail)
            emit_vproj()
            emit_gproj(0)
            emit_gproj(1)
        else:
            emit_vproj()
        # Precompute en*eb for even heads on the otherwise-idle GPSIMD —
        # depends only on the eb load, so it runs ahead of the critical path.
        pw = None
        if PREMUL_HEADS:
            pw = wp.tile(
                [128, len(PREMUL_HEADS), 3, Q], bf, tag="pw", name=f"pw_{b}"
            )
            for i, hx in enumerate(PREMUL_HEADS):
                nc.gpsimd.tensor_tensor(pw[:, i], en_sb[:, hx], eb[:], op=MUL)
        # pair p's (2*sum) rows are DMA'd (engines cannot do partition-strided
        # APs) from the evacuated wa tiles into rows {2p, 2p+1} of sums_bf.
        sums_bf = gp.tile([8, Q], bf, tag="sums_bf", name=f"sums_bf_{b}")
        waA = [
            wap.tile([128, Q], bf, tag="waA", name=f"waA{j}_{b}") for j in range(2)
        ]
        return dict(
            qd=qd, md=md, eb=eb, qT=qT, kT=kT, gt=gt, vv=vv, pw=pw,
            sums_bf=sums_bf, waA=waA, wa_tiles=[], psW=None,
        )

    def emit_heads(b, st, heads):
        qT, kT, eb, vv = st["qT"], st["kT"], st["eb"], st["vv"]
        sums_bf, waA = st["sums_bf"], st["waA"]
        for h in heads:
            j, hh, p = h // 4, h % 4, h % 2
            psS = Sp.tile([128, 1536], f32, tag="S")
            for kc in range(3):
                nc.tensor.matmul(
                    psS[:, 512 * kc : 512 * kc + Q],
                    kT[32 * hh : 32 * (hh + 1), j, 128 * kc : 128 * (kc + 1)],
                    qT[32 * hh : 32 * (hh + 1), j, :],
                    start=True,
                    stop=True,
                    tile_position=(32 * hh, 0),
                )
            sview = psS[:].rearrange("p (c x) -> p c x", x=512)[:, :, :Q]
            if PAIR_MUL:
                if p == 0:
                    st["es2"] = wp.tile(
                        [128, 2, 3, Q], bf, tag="es2", name=f"es2_{h}_{b}"
                    )
                nc.scalar.activation(st["es2"][:, p], sview, Exp)
            else:
                w = wp.tile([128, 3, Q], bf, tag="w", bufs=int(os.environ.get("W_BUFS", "4")))
                if os.environ.get("INPLACE_MUL", "0") == "1":
                    nc.scalar.activation(w[:], sview, Exp)
                    eng = nc.gpsimd if hh in GPS_HEADS else nc.vector
                    eng.tensor_tensor(w[:], w[:], en_sb[:, h], op=MUL)
                    eng.tensor_tensor(w[:], w[:], eb[:], op=MUL)
                elif h in PREMUL_HEADS:
                    es = wp.tile([128, 3, Q], bf, tag="es", bufs=int(os.environ.get("ES_BUFS", "4")))
                    nc.scalar.activation(es[:], sview, Exp)
                    nc.vector.tensor_tensor(
                        w[:], es[:], st["pw"][:, PREMUL_HEADS.index(h)], op=MUL
                    )
                else:
                    es = wp.tile([128, 3, Q], bf, tag="es", bufs=int(os.environ.get("ES_BUFS", "4")))
                    nc.scalar.activation(es[:], sview, Exp)
                    eng = nc.gpsimd if hh in GPS_HEADS else nc.vector
                    eng.tensor_tensor(w[:], es[:], en_sb[:, h], op=MUL)
                    eng.tensor_tensor(w[:], w[:], eb[:], op=MUL)
            if p == 0:
                st["psW"] = sp.tile([128, 512], f32, tag="sm", name=f"psW{h}_{b}")
            psW = st["psW"]
            if PAIR_MUL:
                if p == 1:
                    w2 = wp.tile([128, 2, 3, Q], bf, tag="w2", name=f"w2_{h}_{b}")
                    nc.vector.tensor_tensor(
                        w2[:], st["es2"][:], en_sb[:, h - 1 : h + 1], op=MUL
                    )
                    nc.vector.tensor_tensor(
                        w2[:],
                        w2[:],
                        eb[:].unsqueeze(1).broadcast_to((128, 2, 3, Q)),
                        op=MUL,
                    )
                    for hp in range(2):
                        hx = h - 1 + hp
                        for kc in range(3):
                            nc.tensor.matmul(
                                psW[64 * hp : 64 * hp + 64, :Q],
                                vv[:, kc, 33 * hx : 33 * hx + 64],
                                w2[:, hp, kc, :],
                                start=(kc == 0),
                                stop=(kc == 2),
                            )
            else:
                for kc in range(3):
                    nc.tensor.matmul(
                        psW[64 * p : 64 * p + 64, :Q],
                        vv[:, kc, 33 * h : 33 * h + 64],
                        w[:, kc, :],
                        start=(kc == 0),
                        stop=(kc == 2),
                    )
            if p == 1:
                # evacuate both heads (incl. the 2*sum rows 32 and 96)
                wa = wap.tile(
                    [128, Q],
                    f32 if os.environ.get("WA_F32", "0") == "1" else bf,
                    tag="wa",
                )
                _wa_mode = os.environ.get("WA_EV", "dve")
                if _wa_mode == "act":
                    ev = nc.scalar
                elif _wa_mode == "dve":
                    ev = nc.vector
                else:
                    ev = nc.scalar if (h // 2) % 2 else nc.vector
                if ev is nc.scalar:
                    ev_inst = ev.copy(wa[:, :], psW[:, :Q])
                else:
                    ev_inst = ev.tensor_copy(wa[:, :], psW[:, :Q])
                pr = 2 * (h // 2)
                _dq = nc.scalar if os.environ.get("SDMA_ACT", "0") == "1" else nc.sync
                if os.environ.get("MERGE_DMA", "0") == "1":
                    # merged per-pair DMAs with explicit deps on the evac
                    # (Tile's tracker misses deps for strided partition APs)
                    import bass_rust as _br
                    from concourse.tile_rust import add_dep_helper as _adh

                    d1 = _dq.dma_start(sums_bf[pr : pr + 2, :], wa[32:97:64, :])
                    _adh(d1.ins, ev_inst.ins, reason="strided sums read of wa")
                    jj = h // 4
                    r_dst = 32 * ((h - 1) % 4)
                    srcap = _br.AP(
                        wa.tensor, wa.offset, [[64 * Q, 2], [Q, 32], [1, Q]]
                    )
                    dstap = _br.AP(
                        waA[jj].tensor,
                        waA[jj].offset + r_dst * Q,
                        [[32 * Q, 2], [Q, 32], [1, Q]],
                    )
                    d2 = _dq.dma_start(dstap, srcap)
                    _adh(d2.ins, ev_inst.ins, reason="pair rearrange read of wa")
                else:
                    _dq.dma_start(sums_bf[pr : pr + 1, :], wa[32:33, :])
                    _dq.dma_start(sums_bf[pr + 1 : pr + 2, :], wa[96:97, :])
                    # rearrange both heads into the gate-aligned chunk tile
                    # (engines cannot shift partition base; DMA can)
                    for hx, r0 in ((h - 1, 0), (h, 64)):
                        jj, hh2 = hx // 4, hx % 4
                        _dq.dma_start(
                            waA[jj][32 * hh2 : 32 * hh2 + 32, :], wa[r0 : r0 + 32, :]
                        )
                st["wa_tiles"].append(wa)

    if os.environ.get("LEAD2", "0") == "1":
        # two-batch software-pipeline lead: proj(b+2) is emitted right after
        # heads(b), so every projection has a full head-phase of slack
        states = {0: emit_loads_proj(0), 1: emit_loads_proj(1)}
        prev = None
        for b in range(bpc):
            if prev is not None:
                emit_tail(b - 1, prev["sums_bf"], prev["waA"], prev["gt"])
            st = states.pop(b)
            emit_heads(b, st, range(0, 8))
            if b + 2 < bpc:
                states[b + 2] = emit_loads_proj(b + 2)
            prev = st
        emit_tail(bpc - 1, prev["sums_bf"], prev["waA"], prev["gt"])
    else:
        # Software pipeline: loads+projections of batch b, then the
        # latency-heavy tail of batch b-1 (overlapping this batch's heads).
        prev = None
        for b in range(bpc):
            st = emit_loads_proj(b)
            if prev is not None:
                emit_tail(b - 1, prev["sums_bf"], prev["waA"], prev["gt"])
            emit_heads(b, st, range(0, 8))
            prev = st
        emit_tail(bpc - 1, prev["sums_bf"], prev["waA"], prev["gt"])


def build(bpc=BPC):
    nc = bacc.Bacc(
        "TRN2",
        target_bir_lowering=False,
        debug=False,
        enable_asserts=False,
        num_devices=NUM_CORES,
    )
    f32, bf = mybir.dt.float32, mybir.dt.bfloat16
    io = {
        "inT": nc.dram_tensor("inT", [bpc, 128, 7, Q], bf, kind="ExternalInput").ap(),
        "enT": nc.dram_tensor("enT", [128, H, 3, Q], bf, kind="ExternalInput").ap(),
        "wq": nc.dram_tensor("wq", [128, 2, 256], bf, kind="ExternalInput").ap(),
        "wk": nc.dram_tensor("wk", [128, 2, 256], bf, kind="ExternalInput").ap(),
        "wv": nc.dram_tensor("wv", [128, 2, 256], bf, kind="ExternalInput").ap(),
        "wg": nc.dram_tensor("wg", [128, 2, 256], bf, kind="ExternalInput").ap(),
        "wo": nc.dram_tensor("wo", [128, 2, 256], bf, kind="ExternalInput").ap(),
        "obias_bf": nc.dram_tensor("obias_bf", [1, OUT], bf, kind="ExternalInput").ap(),
        "gbh": nc.dram_tensor("gbh", [128, 2], f32, kind="ExternalInput").ap(),
        "ind": nc.dram_tensor("ind", [128, 256], bf, kind="ExternalInput").ap(),
        "ind_f": nc.dram_tensor("ind_f", [8, 256], f32, kind="ExternalInput").ap(),
        "out": nc.dram_tensor("out", [bpc, Q, OUT], f32, kind="ExternalOutput").ap(),
    }
    with tile.TileContext(nc) as tc:
        with ExitStack() as ctx:
            _build_body(ctx, tc, io, bpc)
    nc.compile()
    return nc


def _prep_inputs(
    q_data,
    m_data,
    bias,
    nonbatched_bias,
    q_weights,
    k_weights,
    v_weights,
    o_weights,
    o_bias,
    gating_w,
    gating_b,
):
    """Host-side preprocessing into the DMA-friendly device layouts."""
    scale = q_weights.shape[-1] ** -0.5

    def featT(x):  # [B, S, A] -> [B, 128, A//128, S]
        b, s, a = x.shape
        t = x.transpose(0, 2, 1).reshape(b, a // 128, 128, s).transpose(0, 2, 1, 3)
        return np.ascontiguousarray(t.astype(BF16))

    qdT = featT(q_data)
    mdT = featT(m_data)
    eb = np.exp(bias[:, 0].transpose(0, 2, 1).astype(np.float32))  # [B, K, Q]
    ebT = np.ascontiguousarray(
        eb.reshape(B, 3, 128, Q).transpose(0, 2, 1, 3).astype(BF16)
    )
    en = np.exp(nonbatched_bias.transpose(0, 2, 1).astype(np.float32))  # [H, K, Q]
    enT = np.ascontiguousarray(
        en.reshape(H, 3, 128, Q).transpose(2, 0, 1, 3).astype(BF16)
    )

    def wmat(w, s=1.0):  # [A, H, hd] -> [128, 2, 256]
        m = (w.reshape(A, H * HD) * s).astype(BF16)
        return np.ascontiguousarray(m.reshape(2, 128, 256).transpose(1, 0, 2))

    wq = wmat(q_weights, scale)
    wk = wmat(k_weights)
    wv = wmat(v_weights)
    wg = wmat(gating_w)
    wo = np.ascontiguousarray(
        o_weights.reshape(256, 256).astype(BF16).reshape(2, 128, 256).transpose(1, 0, 2)
    )
    obias_bf = np.ascontiguousarray(o_bias.astype(BF16).reshape(1, OUT))
    gbh = np.ascontiguousarray(
        (0.5 * gating_b.reshape(H * HD).astype(np.float32)).reshape(2, 128).T
    )
    # indicator for the recip broadcast: row h selects the 32 output
    # partitions belonging to head h.
    ind = np.zeros((128, 256), dtype=BF16)
    for h in range(8):
        ind[h, 32 * h : 32 * (h + 1)] = 1.0
    ind[127, :] = 1.0  # ones row for the o_bias rank-1 matmul
    ind_f = np.ascontiguousarray(ind[0:8].astype(np.float32))
    inT = np.ascontiguousarray(np.concatenate([qdT, mdT, ebT], axis=2))
    return dict(
        inT=inT, enT=enT, wq=wq, wk=wk, wv=wv, wg=wg, wo=wo,
        obias_bf=obias_bf, gbh=gbh, ind=ind, ind_f=ind_f,
    )


_NC_CACHE = {}


def kernel(**inputs):
    from concourse.bass_utils import run_bass_kernel_spmd

    full = _prep_inputs(**{k: np.asarray(v) for k, v in inputs.items()})
    if BPC not in _NC_CACHE:
        _NC_CACHE[BPC] = build(BPC)
    nc = _NC_CACHE[BPC]

    shared = {k: full[k] for k in ("enT", "wq", "wk", "wv", "wg", "wo", "obias_bf", "gbh", "ind", "ind_f")}
    in_maps = []
    for c in range(NUM_CORES):
        sl = slice(c * BPC, (c + 1) * BPC)
        in_maps.append(dict(inT=full["inT"][sl], **shared))

    trace = bool(int(os.environ.get("BASS_KERNEL_TRACE", "0")))
    if trace:
        try:
            from antenv.axon_hooks import get_axon_ntff_profile_hook  # noqa: F401
        except Exception:
            trace = False
    import time

    t0 = time.time()
    res = run_bass_kernel_spmd(
        nc, in_maps, core_ids=list(range(NUM_CORES)), trace=trace
    )
    kernel.last_run_wall_s = time.time() - t0
    if trace and res.exec_time_ns is not None:
        print(f"HW exec time: {res.exec_time_ns} ns")
        kernel.last_exec_time_ns = res.exec_time_ns
    out = np.concatenate([r["out"] for r in res.results], axis=0)
    return out.astype(np.float32)



# revision 32
# speedup vs baseline: 1.0711x; 1.0711x over previous
"""AlphaFold-style gated MSA attention on 8 Trainium2 NeuronCores.

Batch-sharded (128 batches -> 16 per core). Full inputs in, full output out.

Math per batch b (reference):
  q = (q_data @ Wq) * hk^-0.5          [Q, H, 32]
  k = m_data @ Wk ; v = m_data @ Wv    [K, H, 32]
  S[h] = q_h k_h^T + bias[b] + nb[h]   [H, Q, K]
  w = softmax(S, axis=-1)
  wa = w @ v                            [Q, H, 32]
  gate = sigmoid(q_data @ Wg + gb)
  out = (wa * gate).reshape(Q, 256) @ Wo + o_bias

Device-side formulation (per core, per batch):
  All tensors transposed (feature dim on partitions).  S^T[k, q] per head
  from kT/qT projections.  Softmax is unnormalized: the host precomputes
  pw[b, h] = exp(bias[b] + nb[h])^T in bf16 (one DMA per batch) so the
  per-head weights are a single DVE multiply:
      w^T = exp(S^T) * pw[h]
  The V-matmul uses lhsT = [v_h | 2.0] so PSUM row 32/96 accumulates
  2*sum_k w per head (softmax denominators for free).  Head pairs share a
  PSUM bank; evacuated pairs land in wa_all[:, pair, :], from which one
  strided DMA per batch gathers all 8 denominator rows and four per-pair
  DMAs rearrange heads into j-layout waA tiles for the output projection.
  Normalization and gating fuse:
      ga^T = waA * (1 + tanh(x/2 + gb/2)) * recip(2*sum)
  with the per-head recip broadcast across 32 partitions by an indicator
  matmul.  Output projection back to [q, 256] with o_bias added via a
  rank-1 fp8 hi/lo DoubleRow matmul; output stored bf16.
"""

import os
import sys

sys.path.insert(0, "/opt/trn_rl_repo")

import numpy as np
import ml_dtypes
from contextlib import ExitStack

import concourse.bass as bass  # noqa: F401  (engine types)
import concourse.bacc as bacc
import concourse.mybir as mybir
import concourse.tile as tile

BF16 = ml_dtypes.bfloat16
E4M3 = ml_dtypes.float8_e4m3fn

NUM_CORES = 8
B, Q, K, A = 128, 384, 384, 256
H, HD = 8, 32  # heads, head dim
OUT = 256
BPC = B // NUM_CORES  # batches per core


def _env(name, default):
    return os.environ.get(name, default)


def _build_body(ctx, tc, io, bpc):
    nc = tc.nc
    f32, bf = mybir.dt.float32, mybir.dt.bfloat16
    fp8 = mybir.dt.float8e4
    Exp = mybir.ActivationFunctionType.Exp
    Tanh = mybir.ActivationFunctionType.Tanh
    MUL, ADD = mybir.AluOpType.mult, mybir.AluOpType.add
    DR = mybir.MatmulPerfMode.DoubleRow

    const = ctx.enter_context(tc.tile_pool(name="const", bufs=1))
    lp = ctx.enter_context(tc.tile_pool(name="loads", bufs=int(_env("LP_BUFS", "3"))))
    pwp = ctx.enter_context(tc.tile_pool(name="pw", bufs=int(_env("PW_BUFS", "2"))))
    pp = ctx.enter_context(tc.tile_pool(name="proj", bufs=int(_env("PP_BUFS", "4"))))
    wp = ctx.enter_context(tc.tile_pool(name="work", bufs=int(_env("WP_BUFS", "4"))))
    wap = ctx.enter_context(tc.tile_pool(name="wa", bufs=int(_env("WA_BUFS", "2"))))
    gp = ctx.enter_context(tc.tile_pool(name="gating", bufs=int(_env("GP_BUFS", "3"))))
    outp = ctx.enter_context(tc.tile_pool(name="outp", bufs=3))
    # PSUM: 2 x 3 banks (S^T) + 2 x 1 bank (everything else) = 8 banks.
    Sp = ctx.enter_context(tc.tile_pool(name="psum_S", bufs=int(_env("S_BUFS", "2")), space="PSUM"))
    sp = ctx.enter_context(tc.tile_pool(name="psum_sm", bufs=int(_env("SP_BUFS", "2")), space="PSUM"))

    # ---- resident constants ----
    w_sb = {}
    for name in ("wq", "wk", "wv", "wg", "wo"):
        w_sb[name] = const.tile([128, 2, 256], bf, tag=name, name=name)
        nc.sync.dma_start(w_sb[name][:], io[name])
    # o_bias via fp8 hi/lo DoubleRow rank-1: [1,2,128] ones, [1,2,256] bias
    ones2 = const.tile([1, 2, 128], fp8, tag="ones2")
    nc.sync.dma_start(ones2[:], io["ones2"])
    obias2 = const.tile([1, 2, OUT], fp8, tag="obias2")
    nc.sync.dma_start(obias2[:], io["obias2"])
    gbh_sb = const.tile([128, 2], f32, tag="gbh")
    nc.sync.dma_start(gbh_sb[:], io["gbh"])
    if _env("ACT_WARM", "1") == "1":
        # dummy activation right after the first tiny const DMA: pulls the
        # ~1.3us ACT table load off the critical path (exp_and_others holds
        # both Exp and Tanh, so no further loads fire later)
        warm = const.tile([128, 2], f32, tag="warm")
        nc.scalar.activation(warm[:], gbh_sb[:], Exp)
    ind_sb = const.tile([4, 128], bf, tag="ind")
    nc.sync.dma_start(ind_sb[:], io["ind"])
    # duplicated identity for fp8 hi/lo DoubleRow adds of (bias+nb) into S
    idup = const.tile([128, 2, 128], fp8, tag="idup")
    nc.sync.dma_start(idup[:], io["idup"])

    # GPSIMD has no PSUM port: all psum evacuations go ACT or DVE.
    qk_ev = nc.scalar if _env("QK_EV", "dve") == "act" else nc.vector
    vv_ev = nc.scalar if _env("VV_EV", "dve") == "act" else nc.vector
    ob_ev = nc.scalar if _env("OB_EV", "dve") == "act" else nc.vector
    wa_ev = nc.scalar if _env("WA_EV", "dve") == "act" else nc.vector

    def ecopy(eng, dst, src):
        if eng is nc.scalar:
            eng.copy(dst, src)
        else:
            eng.tensor_copy(dst, src)

    from concourse.tile_rust import add_dep_helper as _adh

    wa_war = {}  # buffer-slot -> last strided sums DMA reading that slot
    n_wa_bufs = int(_env("WA_BUFS", "2"))
    states = {}

    def emit_loads(b, ld_pri=None):
        import contextlib
        hp = tc.high_priority() if ld_pri == 0 else contextlib.nullcontext()
        ld = lp.tile([128, 4, Q], bf, tag="ld", name=f"ld_{b}")
        with hp:
            nc.sync.dma_start(ld[:], io["inT"][b])
        cb = pwp.tile([128, H, 3, 2, Q], fp8, tag="cb", name=f"cb_{b}")
        nc.sync.dma_start(cb[:], io["cbT"][b])
        qT = pp.tile([128, 2, Q], bf, tag="qT", name=f"qT_{b}")  # [hc, j, q]
        kT = pp.tile([128, 2, Q], bf, tag="kT", name=f"kT_{b}")  # [hc, j, k]
        gt = pp.tile([128, 2, Q], bf, tag="gt", name=f"gt_{b}")
        # [k, kc, h*33+c | 2.0]; padded to 320 so every head has a 64-wide
        # lhsT window (M=64 writes initialized junk to PSUM rows 32-63,
        # letting the pair evacuation be one full-width copy).
        vv = pp.tile([128, 3, 320], bf, tag="vv", name=f"vv_{b}")
        sums_bf = [
            gp.tile([4, Q], bf, tag=f"sums_bf{i}", name=f"sums_bf{i}_{b}")
            for i in range(2)
        ]
        wa_all = wap.tile([128, 4, Q], bf, tag="wa_all", name=f"wa_all_{b}")
        waA = [
            pp.tile([128, Q], bf, tag=f"waA{j}", name=f"waA{j}_{b}") for j in range(2)
        ]
        states[b] = dict(
            ld=ld, cb=cb, qT=qT, kT=kT, gt=gt, vv=vv,
            sums_bf=sums_bf, wa_all=wa_all, waA=waA, psW=None, psO=None,
            wa_evs=[], ga=[None, None], ob=None, recb=[None, None],
            recb_tc=[None, None],
        )

    def proj_qkg(b, which, j):
        # which: "q" -> qT, "k" -> kT, "g" -> gt (tanh)
        st = states[b]
        ld = st["ld"]
        src_, wname = (ld[:, 0:2, :], "wq") if which == "q" else (
            (ld[:, 2:4, :], "wk") if which == "k" else (ld[:, 0:2, :], "wg")
        )
        ps = sp.tile([128, 512], f32, tag="sm", name=f"ps{which}{j}_{b}")
        for a in range(2):
            nc.tensor.matmul(
                ps[:, :Q],
                w_sb[wname][:, a, 128 * j : 128 * (j + 1)],
                src_[:, a, :],
                start=(a == 0),
                stop=(a == 1),
            )
        if which == "g":
            nc.scalar.activation(
                st["gt"][:, j, :], ps[:, :Q], Tanh, bias=gbh_sb[:, j : j + 1], scale=0.5
            )
        else:
            dst = st["qT"] if which == "q" else st["kT"]
            ecopy(qk_ev, dst[:, j, :], ps[:, :Q])

    def proj_v01(b):
        st = states[b]
        md = st["ld"][:, 2:4, :]
        vv = st["vv"]
        nc.gpsimd.memset(vv[:], 2.0)
        ps = sp.tile([128, 512], f32, tag="sm", name=f"psv01_{b}")
        for kc in range(2):
            for a in range(2):
                nc.tensor.matmul(
                    ps[:, 256 * kc : 256 * (kc + 1)],
                    md[:, a, 128 * kc : 128 * (kc + 1)],
                    w_sb["wv"][:, a, :],
                    start=(a == 0),
                    stop=(a == 1),
                )
        ecopy(
            vv_ev,
            vv[:, 0:2, 0:264].rearrange("p k (h c) -> p k h c", c=33)[:, :, :, 0:32],
            ps[:].rearrange("p (k h c) -> p k h c", k=2, c=32),
        )

    def proj_v2(b):
        st = states[b]
        md = st["ld"][:, 2:4, :]
        ps = sp.tile([128, 512], f32, tag="sm", name=f"psv2_{b}")
        for a in range(2):
            nc.tensor.matmul(
                ps[:, :256],
                md[:, a, 256:384],
                w_sb["wv"][:, a, :],
                start=(a == 0),
                stop=(a == 1),
            )
        ecopy(
            vv_ev,
            st["vv"][:, 2, 0:264].rearrange("p (h c) -> p h c", c=33)[:, :, 0:32],
            ps[:, :256].rearrange("p (h c) -> p h c", c=32),
        )

    PROJ_PIECES = [
        lambda b: proj_qkg(b, "q", 0),
        lambda b: proj_qkg(b, "k", 0),
        lambda b: proj_qkg(b, "q", 1),
        lambda b: proj_qkg(b, "k", 1),
        proj_v01,
        proj_v2,
        lambda b: proj_qkg(b, "g", 0),
        lambda b: proj_qkg(b, "g", 1),
    ]

    def emit_S(b, h):
        st = states[b]
        j, hh = h // 4, h % 4
        psS = Sp.tile([128, 1536], f32, tag="S", name=f"psS{h}_{b}")
        for kc in range(3):
            qk_mm = nc.tensor.matmul(
                psS[:, 512 * kc : 512 * kc + Q],
                st["kT"][32 * hh : 32 * (hh + 1), j, 128 * kc : 128 * (kc + 1)],
                st["qT"][32 * hh : 32 * (hh + 1), j, :],
                start=True,
                stop=False,
                tile_position=(32 * hh, 0),
            )
            # accumulate bias+nb via fp8 hi/lo DoubleRow identity-add
            add_mm = nc.tensor.matmul(
                psS[:, 512 * kc : 512 * kc + Q],
                idup[:],
                st["cb"][:, h, kc, :, :],
                start=False,
                stop=True,
                perf_mode=DR,
            )
            # no data dep links the group members; forbid scheduler reorder
            _adh(add_mm.ins, qk_mm.ins, reason="accumulation group order")
        sview = psS[:].rearrange("p (c x) -> p c x", x=512)[:, :, :Q]
        w = wp.tile([128, 3, Q], bf, tag="w", name=f"w_{h}_{b}",
                    bufs=int(_env("W_BUFS", "6")))
        if _env("EXP_SPLIT", "1") == "2":
            # two ACT ops: frees psS chunks 0-1 for head h+2 sooner
            nc.scalar.activation(w[:, 0:2, :], sview[:, 0:2, :], Exp)
            nc.scalar.activation(w[:, 2, :], sview[:, 2, :], Exp)
        else:
            nc.scalar.activation(w[:], sview, Exp)
        st[f"w{h}"] = w

    def emit_AVpair(b, P):
        st = states[b]
        vv, wa_all, waA = st["vv"], st["wa_all"], st["waA"]
        psW = sp.tile([128, 512], f32, tag="sm", name=f"psW{P}_{b}")
        for p in range(2):
            h = 2 * P + p
            w = st.pop(f"w{h}")
            for kc in range(3):
                nc.tensor.matmul(
                    psW[64 * p : 64 * p + 64, :Q],
                    vv[:, kc, 33 * h : 33 * h + 64],
                    w[:, kc, :],
                    start=(kc == 0),
                    stop=(kc == 2),
                )
        _ev = wa_ev if P < 2 else (
            nc.scalar if _env("WA_EV_LATE", "dve") == "act" else wa_ev
        )
        ev_inst = (
            _ev.copy(wa_all[:, P, :], psW[:, :Q])
            if _ev is nc.scalar
            else _ev.tensor_copy(wa_all[:, P, :], psW[:, :Q])
        )
        st["wa_evs"].append(ev_inst)
        # heads (2P, 2P+1) -> waA[j] rows 64*(P%2) .. +64
        # (contiguous partition ranges both sides: tracker-visible)
        jj = P // 2
        r0 = 64 * (P % 2)
        nc.sync.dma_start(waA[jj][r0 : r0 + 32, :], wa_all[0:32, P, :])
        nc.sync.dma_start(waA[jj][r0 + 32 : r0 + 64, :], wa_all[64:96, P, :])

    def emit_sums(b, half):
        # two DMAs gather the 4 denominator rows for pairs (2*half, 2*half+1):
        # psW row 32 holds head 2P, row 96 head 2P+1.  Contiguous partition
        # ranges on both sides keep the Tile tracker's deps exact.  Row order
        # in sums_bf is (2P, 2P+2, 2P+1, 2P+3); ind compensates.
        st = states[b]
        sb = st["sums_bf"][half]
        wa = st["wa_all"]
        nc.sync.dma_start(sb[0:2, :], wa[32:33, 2 * half : 2 * half + 2, :])
        nc.sync.dma_start(sb[2:4, :], wa[96:97, 2 * half : 2 * half + 2, :])

    def tail_chain(b, half):
        import contextlib
        hp = tc.high_priority(int(_env("CHAIN_PRI", "200"))) if _env(
            "CHAIN_HIPRI", "1"
        ) == "1" else contextlib.nullcontext()
        with hp:
            _tail_chain(b, half)

    def _tail_chain(b, half):
        st = states[b]
        sums_f = gp.tile([4, Q], f32, tag=f"sums_f{half}", name=f"sums_f{half}_{b}")
        nc.vector.tensor_copy(sums_f[:], st["sums_bf"][half][:])
        rec = gp.tile([4, Q], f32, tag=f"rec{half}", name=f"rec{half}_{b}")
        nc.vector.reciprocal_approx_fast(rec[:], sums_f[:])
        recb = gp.tile([4, Q], bf, tag=f"recb{half}", name=f"recb{half}_{b}")
        tci = nc.vector.tensor_copy(recb[:], rec[:])
        st["recb"][half] = recb
        st["recb_tc"][half] = tci

    def tail_gate(b, j):
        st = states[b]
        psR = sp.tile([128, 512], f32, tag="sm", name=f"psR{j}_{b}")
        nc.tensor.matmul(
            psR[:, :Q],
            ind_sb[:],
            st["recb"][j][:],
            start=True,
            stop=True,
        )
        g2 = gp.tile([128, Q], bf, tag="g2", name=f"g2{j}_{b}")
        nc.vector.scalar_tensor_tensor(
            g2[:], st["gt"][:, j, :], 1.0, psR[:, :Q], op0=ADD, op1=MUL
        )
        ga = gp.tile([128, Q], bf, tag="ga", name=f"ga{j}_{b}")
        ga_eng = nc.gpsimd if _env("GA_ENG", "dve") == "gps" else nc.vector
        ga_eng.tensor_tensor(ga[:], st["waA"][j][:], g2[:], op=MUL)
        st["ga"][j] = ga

    def tail_out_mm(b, qc):
        # qc 0..1 share one psum bank (two 256-wide accumulation groups);
        # qc==2 gets its own.  Evacuation happens per 256-wide group when the
        # group stops, keeping bank occupancy short.
        st = states[b]
        if st["ob"] is None:
            st["ob"] = outp.tile([128, 3, OUT], bf, tag="ob", name=f"ob_{b}")
        if qc in (0, 2):
            st["psO"] = sp.tile([128, 512], f32, tag="sm", name=f"psO{qc}_{b}")
        psO = st["psO"]
        off = OUT * (qc % 2)
        for j in range(2):
            nc.tensor.matmul(
                psO[:, off : off + OUT],
                st["ga"][j][:, 128 * qc : 128 * (qc + 1)],
                w_sb["wo"][:, j, :],
                start=(j == 0),
                stop=False,
            )
        # rank-1 accumulate of o_bias (fp8 hi/lo DoubleRow)
        nc.tensor.matmul(
            psO[:, off : off + OUT],
            ones2[:],
            obias2[:],
            start=False,
            stop=True,
            perf_mode=DR,
        )
        ecopy(ob_ev, st["ob"][:, qc, :], psO[:, off : off + OUT])

    def tail_out(b):
        st = states[b]
        nc.sync.dma_start(
            io["out"][b].rearrange("(c p) o -> p c o", p=128), st["ob"][:]
        )

    # ---- slot-woven software pipeline ----
    # slot h of batch b emits: S/exp/mul for head h of b, lagged AV pairs of
    # b (with the first denominator half's recip chain still inside b), the
    # remaining tail of b-1, and proj piece h of b+1.
    emit_loads(0)
    if bpc > 1:
        emit_loads(1)
    for i in range(8):
        PROJ_PIECES[i](0)
    for b in range(bpc):
        for h in range(9):
            if h < 8:
                emit_S(b, h)
            if h == 3:
                emit_AVpair(b, 0)
            elif h == 5:
                emit_AVpair(b, 1)
                emit_sums(b, 0)
            elif h == 6:
                tail_chain(b, 0)
            elif h == 7:
                emit_AVpair(b, 2)
            elif h == 8:
                emit_AVpair(b, 3)
                emit_sums(b, 1)
            if b > 0:
                pb = b - 1
                if h == 0:
                    tail_chain(pb, 1)
                elif h == 1:
                    tail_gate(pb, 0)
                elif h == 2:
                    tail_gate(pb, 1)
                elif h == 3:
                    tail_out_mm(pb, 0)
                elif h == 4:
                    tail_out_mm(pb, 1)
                elif h == 5:
                    tail_out_mm(pb, 2)
                elif h == 6:
                    tail_out(pb)
            if h < 8 and b + 1 < bpc:
                PROJ_PIECES[h](b + 1)
            if h == 4 and b + 2 < bpc:
                emit_loads(b + 2)
        if b - 1 >= 0:
            states.pop(b - 1)
    bl = bpc - 1
    tail_chain(bl, 1)
    tail_gate(bl, 0)
    tail_gate(bl, 1)
    for qc in range(3):
        tail_out_mm(bl, qc)
    tail_out(bl)


def build(bpc=BPC):
    nc = bacc.Bacc(
        "TRN2",
        target_bir_lowering=False,
        debug=False,
        enable_asserts=False,
        num_devices=NUM_CORES,
    )
    f32, bf = mybir.dt.float32, mybir.dt.bfloat16
    fp8 = mybir.dt.float8e4
    io = {
        "inT": nc.dram_tensor("inT", [bpc, 128, 4, Q], bf, kind="ExternalInput").ap(),
        "cbT": nc.dram_tensor(
            "cbT", [bpc, 128, H, 3, 2, Q], fp8, kind="ExternalInput"
        ).ap(),
        "idup": nc.dram_tensor("idup", [128, 2, 128], fp8, kind="ExternalInput").ap(),
        "wq": nc.dram_tensor("wq", [128, 2, 256], bf, kind="ExternalInput").ap(),
        "wk": nc.dram_tensor("wk", [128, 2, 256], bf, kind="ExternalInput").ap(),
        "wv": nc.dram_tensor("wv", [128, 2, 256], bf, kind="ExternalInput").ap(),
        "wg": nc.dram_tensor("wg", [128, 2, 256], bf, kind="ExternalInput").ap(),
        "wo": nc.dram_tensor("wo", [128, 2, 256], bf, kind="ExternalInput").ap(),
        "ones2": nc.dram_tensor("ones2", [1, 2, 128], fp8, kind="ExternalInput").ap(),
        "obias2": nc.dram_tensor("obias2", [1, 2, OUT], fp8, kind="ExternalInput").ap(),
        "gbh": nc.dram_tensor("gbh", [128, 2], f32, kind="ExternalInput").ap(),
        "ind": nc.dram_tensor("ind", [4, 128], bf, kind="ExternalInput").ap(),
        "out": nc.dram_tensor("out", [bpc, Q, OUT], bf, kind="ExternalOutput").ap(),
    }
    with tile.TileContext(nc) as tc:
        with ExitStack() as ctx:
            _build_body(ctx, tc, io, bpc)
    nc.compile()
    return nc


def _prep_inputs(
    q_data,
    m_data,
    bias,
    nonbatched_bias,
    q_weights,
    k_weights,
    v_weights,
    o_weights,
    o_bias,
    gating_w,
    gating_b,
):
    """Host-side preprocessing into the DMA-friendly device layouts."""
    scale = q_weights.shape[-1] ** -0.5

    def featT(x):  # [B, S, A] -> [B, 128, A//128, S]
        b, s, a = x.shape
        t = x.transpose(0, 2, 1).reshape(b, a // 128, 128, s).transpose(0, 2, 1, 3)
        return np.ascontiguousarray(t.astype(BF16))

    qdT = featT(q_data)
    mdT = featT(m_data)
    inT = np.ascontiguousarray(np.concatenate([qdT, mdT], axis=2))

    # cb[b, p, h, kc, t, q] = hi/lo e4m3 split of
    #   bias[b, q, 128*kc+p] + nb[h, q, 128*kc+p]
    bT = bias[:, 0].transpose(0, 2, 1).astype(np.float32)  # [B, K, Q]
    nT = nonbatched_bias.transpose(0, 2, 1).astype(np.float32)  # [H, K, Q]
    comb = bT[:, None] + nT[None]  # [B, H, K, Q]
    hi = comb.astype(E4M3)
    lo = (comb - hi.astype(np.float32)).astype(E4M3)
    cbT = np.stack([hi, lo], axis=-2)  # [B, H, K, 2, Q]
    cbT = np.ascontiguousarray(
        cbT.reshape(B, H, 3, 128, 2, Q).transpose(0, 3, 1, 2, 4, 5)
    )
    idup = np.zeros((128, 2, 128), dtype=E4M3)
    for t in range(2):
        np.fill_diagonal(idup[:, t, :], 1.0)

    def wmat(w, s=1.0):  # [A, H, hd] -> [128, 2, 256]
        m = (w.reshape(A, H * HD) * s).astype(BF16)
        return np.ascontiguousarray(m.reshape(2, 128, 256).transpose(1, 0, 2))

    wq = wmat(q_weights, scale)
    wk = wmat(k_weights)
    wv = wmat(v_weights)
    wg = wmat(gating_w)
    wo = np.ascontiguousarray(
        o_weights.reshape(256, 256).astype(BF16).reshape(2, 128, 256).transpose(1, 0, 2)
    )
    # o_bias rank-1 via fp8 hi/lo DoubleRow: ones2.T @ obias2 = 16*(hi+lo)/16
    # scale bias up x16 before fp8 split, ones row = 1/16 (exact in fp8)
    obf = o_bias.astype(np.float32) * 16.0
    hi = np.clip(obf, -240, 240).astype(E4M3)
    lo = np.clip(obf - hi.astype(np.float32), -240, 240).astype(E4M3)
    obias2 = np.ascontiguousarray(np.stack([hi, lo]).reshape(1, 2, OUT))
    ones2 = np.full((1, 2, 128), 1.0 / 16.0, dtype=E4M3)
    gbh = np.ascontiguousarray(
        (0.5 * gating_b.reshape(H * HD).astype(np.float32)).reshape(2, 128).T
    )
    # indicator for the recip broadcast: sums rows are pair-major = head
    # order within each j-group, so one [4,128] block serves both j's
    ind = np.zeros((4, 128), dtype=BF16)
    for r, hh in enumerate((0, 2, 1, 3)):
        ind[r, 32 * hh : 32 * (hh + 1)] = 1.0
    return dict(
        inT=inT, cbT=cbT, wq=wq, wk=wk, wv=wv, wg=wg, wo=wo,
        ones2=ones2, obias2=obias2, gbh=gbh, ind=ind, idup=idup,
    )


_NC_CACHE = {}


def kernel(**inputs):
    from concourse.bass_utils import run_bass_kernel_spmd

    full = _prep_inputs(**{k: np.asarray(v) for k, v in inputs.items()})
    if BPC not in _NC_CACHE:
        _NC_CACHE[BPC] = build(BPC)
    nc = _NC_CACHE[BPC]

    shared = {
        k: full[k]
        for k in ("wq", "wk", "wv", "wg", "wo", "ones2", "obias2", "gbh", "ind", "idup")
    }
    in_maps = []
    for c in range(NUM_CORES):
        sl = slice(c * BPC, (c + 1) * BPC)
        in_maps.append(dict(inT=full["inT"][sl], cbT=full["cbT"][sl], **shared))

    trace = bool(int(os.environ.get("BASS_KERNEL_TRACE", "0")))
    if trace:
        try:
            from antenv.axon_hooks import get_axon_ntff_profile_hook  # noqa: F401
        except Exception:
            trace = False
    import time

    t0 = time.time()
    res = run_bass_kernel_spmd(
        nc, in_maps, core_ids=list(range(NUM_CORES)), trace=trace
    )
    kernel.last_run_wall_s = time.time() - t0
    if trace and res.exec_time_ns is not None:
        print(f"HW exec time: {res.exec_time_ns} ns")
        kernel.last_exec_time_ns = res.exec_time_ns
    out = np.concatenate([r["out"] for r in res.results], axis=0)
    return out.astype(np.float32)


# revision 37
# speedup vs baseline: 1.1093x; 1.0357x over previous
"""AlphaFold-style gated MSA attention on 8 Trainium2 NeuronCores.

Batch-sharded (128 batches -> 16 per core). Full inputs in, full output out.

Math per batch b (reference):
  q = (q_data @ Wq) * hk^-0.5          [Q, H, 32]
  k = m_data @ Wk ; v = m_data @ Wv    [K, H, 32]
  S[h] = q_h k_h^T + bias[b] + nb[h]   [H, Q, K]
  w = softmax(S, axis=-1)
  wa = w @ v                            [Q, H, 32]
  gate = sigmoid(q_data @ Wg + gb)
  out = (wa * gate).reshape(Q, 256) @ Wo + o_bias

Device-side formulation (per core, per batch):
  All tensors transposed (feature dim on partitions).  S^T[k, q] per head
  from kT/qT projections.  Softmax is unnormalized: the host precomputes
  pw[b, h] = exp(bias[b] + nb[h])^T in bf16 (one DMA per batch) so the
  per-head weights are a single DVE multiply:
      w^T = exp(S^T) * pw[h]
  The V-matmul uses lhsT = [v_h | 2.0] so PSUM row 32/96 accumulates
  2*sum_k w per head (softmax denominators for free).  Head pairs share a
  PSUM bank; evacuated pairs land in wa_all[:, pair, :], from which one
  strided DMA per batch gathers all 8 denominator rows and four per-pair
  DMAs rearrange heads into j-layout waA tiles for the output projection.
  Normalization and gating fuse:
      ga^T = waA * (1 + tanh(x/2 + gb/2)) * recip(2*sum)
  with the per-head recip broadcast across 32 partitions by an indicator
  matmul.  Output projection back to [q, 256] with o_bias added via a
  rank-1 fp8 hi/lo DoubleRow matmul; output stored bf16.
"""

import os
import sys

sys.path.insert(0, "/opt/trn_rl_repo")

import numpy as np
import ml_dtypes
from contextlib import ExitStack

import concourse.bass as bass  # noqa: F401  (engine types)
import concourse.bacc as bacc
import concourse.mybir as mybir
import concourse.tile as tile

BF16 = ml_dtypes.bfloat16
E4M3 = ml_dtypes.float8_e4m3fn

NUM_CORES = 8
B, Q, K, A = 128, 384, 384, 256
H, HD = 8, 32  # heads, head dim
OUT = 256
BPC = B // NUM_CORES  # batches per core


def _env(name, default):
    return os.environ.get(name, default)


def _build_body(ctx, tc, io, bpc):
    nc = tc.nc
    f32, bf = mybir.dt.float32, mybir.dt.bfloat16
    fp8 = mybir.dt.float8e4
    Exp = mybir.ActivationFunctionType.Exp
    Tanh = mybir.ActivationFunctionType.Tanh
    MUL, ADD = mybir.AluOpType.mult, mybir.AluOpType.add
    DR = mybir.MatmulPerfMode.DoubleRow

    const = ctx.enter_context(tc.tile_pool(name="const", bufs=1))
    lp = ctx.enter_context(tc.tile_pool(name="loads", bufs=int(_env("LP_BUFS", "3"))))
    pwp = ctx.enter_context(tc.tile_pool(name="pw", bufs=int(_env("PW_BUFS", "2"))))
    pp = ctx.enter_context(tc.tile_pool(name="proj", bufs=int(_env("PP_BUFS", "4"))))
    wp = ctx.enter_context(tc.tile_pool(name="work", bufs=int(_env("WP_BUFS", "4"))))
    wap = ctx.enter_context(tc.tile_pool(name="wa", bufs=int(_env("WA_BUFS", "2"))))
    gp = ctx.enter_context(tc.tile_pool(name="gating", bufs=int(_env("GP_BUFS", "3"))))
    outp = ctx.enter_context(tc.tile_pool(name="outp", bufs=3))
    # PSUM: 2 x 3 banks (S^T) + 2 x 1 bank (everything else) = 8 banks.
    Sp = ctx.enter_context(tc.tile_pool(name="psum_S", bufs=int(_env("S_BUFS", "2")), space="PSUM"))
    sp = ctx.enter_context(tc.tile_pool(name="psum_sm", bufs=int(_env("SP_BUFS", "2")), space="PSUM"))

    # ---- resident constants ----
    w_sb = {}
    for name in ("wq", "wk", "wv", "wg", "wo"):
        w_sb[name] = const.tile([128, 2, 256], bf, tag=name, name=name)
        nc.sync.dma_start(w_sb[name][:], io[name])
    # o_bias via fp8 hi/lo DoubleRow rank-1: [1,2,128] ones, [1,2,256] bias
    ones2 = const.tile([1, 2, 128], fp8, tag="ones2")
    nc.sync.dma_start(ones2[:], io["ones2"])
    obias2 = const.tile([1, 2, OUT], fp8, tag="obias2")
    nc.sync.dma_start(obias2[:], io["obias2"])
    gbh_sb = const.tile([128, 2], f32, tag="gbh")
    nc.sync.dma_start(gbh_sb[:], io["gbh"])
    if _env("ACT_WARM", "1") == "1":
        # dummy activation right after the first tiny const DMA: pulls the
        # ~1.3us ACT table load off the critical path (exp_and_others holds
        # both Exp and Tanh, so no further loads fire later)
        warm = const.tile([128, 2], f32, tag="warm")
        nc.scalar.activation(warm[:], gbh_sb[:], Exp)
    ind_sb = const.tile([4, 128], bf, tag="ind")
    nc.sync.dma_start(ind_sb[:], io["ind"])
    # duplicated identity for fp8 hi/lo DoubleRow adds of (bias+nb) into S
    idup = const.tile([128, 2, 128], fp8, tag="idup")
    nc.sync.dma_start(idup[:], io["idup"])

    # GPSIMD has no PSUM port: all psum evacuations go ACT or DVE.
    qk_ev = nc.scalar if _env("QK_EV", "dve") == "act" else nc.vector
    vv_ev = nc.scalar if _env("VV_EV", "dve") == "act" else nc.vector
    ob_ev = nc.scalar if _env("OB_EV", "dve") == "act" else nc.vector
    wa_ev = nc.scalar if _env("WA_EV", "dve") == "act" else nc.vector

    def ecopy(eng, dst, src):
        if eng is nc.scalar:
            eng.copy(dst, src)
        else:
            eng.tensor_copy(dst, src)

    from concourse.tile_rust import add_dep_helper as _adh

    wa_war = {}  # buffer-slot -> last strided sums DMA reading that slot
    n_wa_bufs = int(_env("WA_BUFS", "2"))
    states = {}

    def emit_loads(b, ld_pri=None):
        import contextlib
        hp = tc.high_priority() if ld_pri == 0 else contextlib.nullcontext()
        ld = lp.tile([128, 4, Q], bf, tag="ld", name=f"ld_{b}")
        with hp:
            nc.sync.dma_start(ld[:], io["inT"][b])
        cb = pwp.tile([128, H, 3, 2, Q], fp8, tag="cb", name=f"cb_{b}")
        nc.sync.dma_start(cb[:], io["cbT"][b])
        qT = pp.tile([128, 2, Q], bf, tag="qT", name=f"qT_{b}")  # [hc, j, q]
        kT = pp.tile([128, 2, Q], bf, tag="kT", name=f"kT_{b}")  # [hc, j, k]
        gt = pp.tile([128, 2, Q], bf, tag="gt", name=f"gt_{b}")
        # [k, kc, h*33+c | 2.0]; padded to 320 so every head has a 64-wide
        # lhsT window (M=64 writes initialized junk to PSUM rows 32-63,
        # letting the pair evacuation be one full-width copy).
        vv = pp.tile([128, 3, 320], bf, tag="vv", name=f"vv_{b}")
        sums_bf = [
            gp.tile([4, Q], bf, tag=f"sums_bf{i}", name=f"sums_bf{i}_{b}")
            for i in range(2)
        ]
        wa_all = wap.tile([128, 4, Q], bf, tag="wa_all", name=f"wa_all_{b}")
        waA = [
            pp.tile([128, Q], bf, tag=f"waA{j}", name=f"waA{j}_{b}") for j in range(2)
        ]
        states[b] = dict(
            ld=ld, cb=cb, qT=qT, kT=kT, gt=gt, vv=vv,
            sums_bf=sums_bf, wa_all=wa_all, waA=waA, psW=None, psO=None,
            wa_evs=[], ga=[None, None], ob=None, recb=[None, None],
            recb_tc=[None, None],
        )

    def proj_qkg(b, which, j):
        # which: "q" -> qT, "k" -> kT, "g" -> gt (tanh)
        st = states[b]
        ld = st["ld"]
        src_, wname = (ld[:, 0:2, :], "wq") if which == "q" else (
            (ld[:, 2:4, :], "wk") if which == "k" else (ld[:, 0:2, :], "wg")
        )
        ps = sp.tile([128, 512], f32, tag="sm", name=f"ps{which}{j}_{b}")
        for a in range(2):
            nc.tensor.matmul(
                ps[:, :Q],
                w_sb[wname][:, a, 128 * j : 128 * (j + 1)],
                src_[:, a, :],
                start=(a == 0),
                stop=(a == 1),
            )
        if which == "g":
            nc.scalar.activation(
                st["gt"][:, j, :], ps[:, :Q], Tanh, bias=gbh_sb[:, j : j + 1], scale=0.5
            )
        else:
            dst = st["qT"] if which == "q" else st["kT"]
            ecopy(qk_ev, dst[:, j, :], ps[:, :Q])

    def proj_v01(b):
        st = states[b]
        md = st["ld"][:, 2:4, :]
        vv = st["vv"]
        nc.gpsimd.memset(vv[:], 2.0)
        ps = sp.tile([128, 512], f32, tag="sm", name=f"psv01_{b}")
        for kc in range(2):
            for a in range(2):
                nc.tensor.matmul(
                    ps[:, 256 * kc : 256 * (kc + 1)],
                    md[:, a, 128 * kc : 128 * (kc + 1)],
                    w_sb["wv"][:, a, :],
                    start=(a == 0),
                    stop=(a == 1),
                )
        ecopy(
            vv_ev,
            vv[:, 0:2, 0:264].rearrange("p k (h c) -> p k h c", c=33)[:, :, :, 0:32],
            ps[:].rearrange("p (k h c) -> p k h c", k=2, c=32),
        )

    def proj_v2(b):
        st = states[b]
        md = st["ld"][:, 2:4, :]
        ps = sp.tile([128, 512], f32, tag="sm", name=f"psv2_{b}")
        for a in range(2):
            nc.tensor.matmul(
                ps[:, :256],
                md[:, a, 256:384],
                w_sb["wv"][:, a, :],
                start=(a == 0),
                stop=(a == 1),
            )
        ecopy(
            vv_ev,
            st["vv"][:, 2, 0:264].rearrange("p (h c) -> p h c", c=33)[:, :, 0:32],
            ps[:, :256].rearrange("p (h c) -> p h c", c=32),
        )

    PROJ_PIECES = [
        lambda b: proj_qkg(b, "q", 0),
        lambda b: proj_qkg(b, "k", 0),
        lambda b: proj_qkg(b, "q", 1),
        lambda b: proj_qkg(b, "k", 1),
        proj_v01,
        proj_v2,
        lambda b: proj_qkg(b, "g", 0),
        lambda b: proj_qkg(b, "g", 1),
    ]

    def emit_S(b, h):
        st = states[b]
        j, hh = h // 4, h % 4
        psS = Sp.tile([128, 1536], f32, tag="S", name=f"psS{h}_{b}")
        for kc in range(3):
            qk_mm = nc.tensor.matmul(
                psS[:, 512 * kc : 512 * kc + Q],
                st["kT"][32 * hh : 32 * (hh + 1), j, 128 * kc : 128 * (kc + 1)],
                st["qT"][32 * hh : 32 * (hh + 1), j, :],
                start=True,
                stop=False,
                tile_position=(32 * hh, 0),
            )
            # accumulate bias+nb via fp8 hi/lo DoubleRow identity-add
            add_mm = nc.tensor.matmul(
                psS[:, 512 * kc : 512 * kc + Q],
                idup[:],
                st["cb"][:, h, kc, :, :],
                start=False,
                stop=True,
                perf_mode=DR,
            )
            # no data dep links the group members; forbid scheduler reorder
            _adh(add_mm.ins, qk_mm.ins, reason="accumulation group order")
        sview = psS[:].rearrange("p (c x) -> p c x", x=512)[:, :, :Q]
        w = wp.tile([128, 3, Q], bf, tag="w", name=f"w_{h}_{b}",
                    bufs=int(_env("W_BUFS", "9")))
        if _env("EXP_SPLIT", "1") == "2":
            # two ACT ops: frees psS chunks 0-1 for head h+2 sooner
            nc.scalar.activation(w[:, 0:2, :], sview[:, 0:2, :], Exp)
            nc.scalar.activation(w[:, 2, :], sview[:, 2, :], Exp)
        else:
            nc.scalar.activation(w[:], sview, Exp)
        st[f"w{h}"] = w

    def emit_AVpair(b, P):
        st = states[b]
        vv, wa_all, waA = st["vv"], st["wa_all"], st["waA"]
        psW = sp.tile([128, 512], f32, tag="sm", name=f"psW{P}_{b}")
        for p in range(2):
            h = 2 * P + p
            w = st.pop(f"w{h}")
            for kc in range(3):
                nc.tensor.matmul(
                    psW[64 * p : 64 * p + 64, :Q],
                    vv[:, kc, 33 * h : 33 * h + 64],
                    w[:, kc, :],
                    start=(kc == 0),
                    stop=(kc == 2),
                )
        _ev = wa_ev if P < 2 else (
            nc.scalar if _env("WA_EV_LATE", "dve") == "act" else wa_ev
        )
        ev_inst = (
            _ev.copy(wa_all[:, P, :], psW[:, :Q])
            if _ev is nc.scalar
            else _ev.tensor_copy(wa_all[:, P, :], psW[:, :Q])
        )
        st["wa_evs"].append(ev_inst)
        # heads (2P, 2P+1) -> waA[j] rows 64*(P%2) .. +64
        # (contiguous partition ranges both sides: tracker-visible)
        jj = P // 2
        r0 = 64 * (P % 2)
        nc.sync.dma_start(waA[jj][r0 : r0 + 32, :], wa_all[0:32, P, :])
        nc.sync.dma_start(waA[jj][r0 + 32 : r0 + 64, :], wa_all[64:96, P, :])

    def emit_sums(b, half):
        # two DMAs gather the 4 denominator rows for pairs (2*half, 2*half+1):
        # psW row 32 holds head 2P, row 96 head 2P+1.  Contiguous partition
        # ranges on both sides keep the Tile tracker's deps exact.  Row order
        # in sums_bf is (2P, 2P+2, 2P+1, 2P+3); ind compensates.
        st = states[b]
        sb = st["sums_bf"][half]
        wa = st["wa_all"]
        nc.sync.dma_start(sb[0:2, :], wa[32:33, 2 * half : 2 * half + 2, :])
        nc.sync.dma_start(sb[2:4, :], wa[96:97, 2 * half : 2 * half + 2, :])

    def tail_chain(b, half):
        import contextlib
        hp = tc.high_priority(int(_env("CHAIN_PRI", "200"))) if _env(
            "CHAIN_HIPRI", "1"
        ) == "1" else contextlib.nullcontext()
        with hp:
            _tail_chain(b, half)

    def _tail_chain(b, half):
        st = states[b]
        sums_f = gp.tile([4, Q], f32, tag=f"sums_f{half}", name=f"sums_f{half}_{b}")
        nc.vector.tensor_copy(sums_f[:], st["sums_bf"][half][:])
        rec = gp.tile([4, Q], f32, tag=f"rec{half}", name=f"rec{half}_{b}")
        nc.vector.reciprocal_approx_fast(rec[:], sums_f[:])
        recb = gp.tile([4, Q], bf, tag=f"recb{half}", name=f"recb{half}_{b}")
        tci = nc.vector.tensor_copy(recb[:], rec[:])
        st["recb"][half] = recb
        st["recb_tc"][half] = tci

    def tail_gate(b, j):
        st = states[b]
        psR = sp.tile([128, 512], f32, tag="sm", name=f"psR{j}_{b}")
        nc.tensor.matmul(
            psR[:, :Q],
            ind_sb[:],
            st["recb"][j][:],
            start=True,
            stop=True,
        )
        g2 = gp.tile([128, Q], bf, tag="g2", name=f"g2{j}_{b}")
        nc.vector.scalar_tensor_tensor(
            g2[:], st["gt"][:, j, :], 1.0, psR[:, :Q], op0=ADD, op1=MUL
        )
        ga = gp.tile([128, Q], bf, tag="ga", name=f"ga{j}_{b}")
        ga_eng = nc.gpsimd if _env("GA_ENG", "dve") == "gps" else nc.vector
        ga_eng.tensor_tensor(ga[:], st["waA"][j][:], g2[:], op=MUL)
        st["ga"][j] = ga

    def tail_out_mm(b, qc):
        # qc 0..1 share one psum bank (two 256-wide accumulation groups);
        # qc==2 gets its own.  Evacuation happens per 256-wide group when the
        # group stops, keeping bank occupancy short.
        st = states[b]
        if st["ob"] is None:
            st["ob"] = outp.tile([128, 3, OUT], bf, tag="ob", name=f"ob_{b}")
        if qc in (0, 2):
            st["psO"] = sp.tile([128, 512], f32, tag="sm", name=f"psO{qc}_{b}")
        psO = st["psO"]
        off = OUT * (qc % 2)
        for j in range(2):
            nc.tensor.matmul(
                psO[:, off : off + OUT],
                st["ga"][j][:, 128 * qc : 128 * (qc + 1)],
                w_sb["wo"][:, j, :],
                start=(j == 0),
                stop=False,
            )
        # rank-1 accumulate of o_bias (fp8 hi/lo DoubleRow)
        nc.tensor.matmul(
            psO[:, off : off + OUT],
            ones2[:],
            obias2[:],
            start=False,
            stop=True,
            perf_mode=DR,
        )
        ecopy(ob_ev, st["ob"][:, qc, :], psO[:, off : off + OUT])

    def tail_out(b):
        st = states[b]
        nc.sync.dma_start(
            io["out"][b].rearrange("(c p) o -> p c o", p=128), st["ob"][:]
        )

    # ---- slot-woven software pipeline ----
    # slot h of batch b emits: S/exp/mul for head h of b, lagged AV pairs of
    # b (with the first denominator half's recip chain still inside b), the
    # remaining tail of b-1, and proj piece h of b+1.
    emit_loads(0)
    if bpc > 1:
        emit_loads(1)
    for i in range(8):
        PROJ_PIECES[i](0)
    for b in range(bpc):
        for h in range(9):
            if h < 8:
                emit_S(b, h)
            if h == 2:
                emit_AVpair(b, 0)
            elif h == 4:
                emit_AVpair(b, 1)
                emit_sums(b, 0)
            elif h == 5:
                tail_chain(b, 0)
            elif h == 7:
                emit_AVpair(b, 2)
            elif h == 8:
                emit_AVpair(b, 3)
                emit_sums(b, 1)
            if b > 0:
                pb = b - 1
                if h == 0:
                    tail_chain(pb, 1)
                elif h == 1:
                    tail_gate(pb, 0)
                elif h == 2:
                    tail_gate(pb, 1)
                elif h == 3:
                    tail_out_mm(pb, 0)
                elif h == 4:
                    tail_out_mm(pb, 1)
                elif h == 5:
                    tail_out_mm(pb, 2)
                elif h == 6:
                    tail_out(pb)
            if h < 8 and b + 1 < bpc:
                PROJ_PIECES[h](b + 1)
            if h == 4 and b + 2 < bpc:
                emit_loads(b + 2)
        if b - 1 >= 0:
            states.pop(b - 1)
    bl = bpc - 1
    tail_chain(bl, 1)
    tail_gate(bl, 0)
    tail_gate(bl, 1)
    for qc in range(3):
        tail_out_mm(bl, qc)
    tail_out(bl)


def build(bpc=BPC):
    nc = bacc.Bacc(
        "TRN2",
        target_bir_lowering=False,
        debug=False,
        enable_asserts=False,
        num_devices=NUM_CORES,
    )
    f32, bf = mybir.dt.float32, mybir.dt.bfloat16
    fp8 = mybir.dt.float8e4
    io = {
        "inT": nc.dram_tensor("inT", [bpc, 128, 4, Q], bf, kind="ExternalInput").ap(),
        "cbT": nc.dram_tensor(
            "cbT", [bpc, 128, H, 3, 2, Q], fp8, kind="ExternalInput"
        ).ap(),
        "idup": nc.dram_tensor("idup", [128, 2, 128], fp8, kind="ExternalInput").ap(),
        "wq": nc.dram_tensor("wq", [128, 2, 256], bf, kind="ExternalInput").ap(),
        "wk": nc.dram_tensor("wk", [128, 2, 256], bf, kind="ExternalInput").ap(),
        "wv": nc.dram_tensor("wv", [128, 2, 256], bf, kind="ExternalInput").ap(),
        "wg": nc.dram_tensor("wg", [128, 2, 256], bf, kind="ExternalInput").ap(),
        "wo": nc.dram_tensor("wo", [128, 2, 256], bf, kind="ExternalInput").ap(),
        "ones2": nc.dram_tensor("ones2", [1, 2, 128], fp8, kind="ExternalInput").ap(),
        "obias2": nc.dram_tensor("obias2", [1, 2, OUT], fp8, kind="ExternalInput").ap(),
        "gbh": nc.dram_tensor("gbh", [128, 2], f32, kind="ExternalInput").ap(),
        "ind": nc.dram_tensor("ind", [4, 128], bf, kind="ExternalInput").ap(),
        "out": nc.dram_tensor("out", [bpc, Q, OUT], bf, kind="ExternalOutput").ap(),
    }
    with tile.TileContext(nc) as tc:
        with ExitStack() as ctx:
            _build_body(ctx, tc, io, bpc)
    nc.compile()
    return nc


def _prep_inputs(
    q_data,
    m_data,
    bias,
    nonbatched_bias,
    q_weights,
    k_weights,
    v_weights,
    o_weights,
    o_bias,
    gating_w,
    gating_b,
):
    """Host-side preprocessing into the DMA-friendly device layouts."""
    scale = q_weights.shape[-1] ** -0.5

    def featT(x):  # [B, S, A] -> [B, 128, A//128, S]
        b, s, a = x.shape
        t = x.transpose(0, 2, 1).reshape(b, a // 128, 128, s).transpose(0, 2, 1, 3)
        return np.ascontiguousarray(t.astype(BF16))

    qdT = featT(q_data)
    mdT = featT(m_data)
    inT = np.ascontiguousarray(np.concatenate([qdT, mdT], axis=2))

    # cb[b, p, h, kc, t, q] = hi/lo e4m3 split of
    #   bias[b, q, 128*kc+p] + nb[h, q, 128*kc+p]
    bT = bias[:, 0].transpose(0, 2, 1).astype(np.float32)  # [B, K, Q]
    nT = nonbatched_bias.transpose(0, 2, 1).astype(np.float32)  # [H, K, Q]
    comb = bT[:, None] + nT[None]  # [B, H, K, Q]
    hi = comb.astype(E4M3)
    lo = (comb - hi.astype(np.float32)).astype(E4M3)
    cbT = np.stack([hi, lo], axis=-2)  # [B, H, K, 2, Q]
    cbT = np.ascontiguousarray(
        cbT.reshape(B, H, 3, 128, 2, Q).transpose(0, 3, 1, 2, 4, 5)
    )
    idup = np.zeros((128, 2, 128), dtype=E4M3)
    for t in range(2):
        np.fill_diagonal(idup[:, t, :], 1.0)

    def wmat(w, s=1.0):  # [A, H, hd] -> [128, 2, 256]
        m = (w.reshape(A, H * HD) * s).astype(BF16)
        return np.ascontiguousarray(m.reshape(2, 128, 256).transpose(1, 0, 2))

    wq = wmat(q_weights, scale)
    wk = wmat(k_weights)
    wv = wmat(v_weights)
    wg = wmat(gating_w)
    wo = np.ascontiguousarray(
        o_weights.reshape(256, 256).astype(BF16).reshape(2, 128, 256).transpose(1, 0, 2)
    )
    # o_bias rank-1 via fp8 hi/lo DoubleRow: ones2.T @ obias2 = 16*(hi+lo)/16
    # scale bias up x16 before fp8 split, ones row = 1/16 (exact in fp8)
    obf = o_bias.astype(np.float32) * 16.0
    hi = np.clip(obf, -240, 240).astype(E4M3)
    lo = np.clip(obf - hi.astype(np.float32), -240, 240).astype(E4M3)
    obias2 = np.ascontiguousarray(np.stack([hi, lo]).reshape(1, 2, OUT))
    ones2 = np.full((1, 2, 128), 1.0 / 16.0, dtype=E4M3)
    gbh = np.ascontiguousarray(
        (0.5 * gating_b.reshape(H * HD).astype(np.float32)).reshape(2, 128).T
    )
    # indicator for the recip broadcast: sums rows are pair-major = head
    # order within each j-group, so one [4,128] block serves both j's
    ind = np.zeros((4, 128), dtype=BF16)
    for r, hh in enumerate((0, 2, 1, 3)):
        ind[r, 32 * hh : 32 * (hh + 1)] = 1.0
    return dict(
        inT=inT, cbT=cbT, wq=wq, wk=wk, wv=wv, wg=wg, wo=wo,
        ones2=ones2, obias2=obias2, gbh=gbh, ind=ind, idup=idup,
    )


_NC_CACHE = {}


def kernel(**inputs):
    from concourse.bass_utils import run_bass_kernel_spmd

    full = _prep_inputs(**{k: np.asarray(v) for k, v in inputs.items()})
    if BPC not in _NC_CACHE:
        _NC_CACHE[BPC] = build(BPC)
    nc = _NC_CACHE[BPC]

    shared = {
        k: full[k]
        for k in ("wq", "wk", "wv", "wg", "wo", "ones2", "obias2", "gbh", "ind", "idup")
    }
    in_maps = []
    for c in range(NUM_CORES):
        sl = slice(c * BPC, (c + 1) * BPC)
        in_maps.append(dict(inT=full["inT"][sl], cbT=full["cbT"][sl], **shared))

    trace = bool(int(os.environ.get("BASS_KERNEL_TRACE", "0")))
    if trace:
        try:
            from antenv.axon_hooks import get_axon_ntff_profile_hook  # noqa: F401
        except Exception:
            trace = False
    import time

    t0 = time.time()
    res = run_bass_kernel_spmd(
        nc, in_maps, core_ids=list(range(NUM_CORES)), trace=trace
    )
    kernel.last_run_wall_s = time.time() - t0
    if trace and res.exec_time_ns is not None:
        print(f"HW exec time: {res.exec_time_ns} ns")
        kernel.last_exec_time_ns = res.exec_time_ns
    out = np.concatenate([r["out"] for r in res.results], axis=0)
    return out.astype(np.float32)


# revision 42
# speedup vs baseline: 1.1121x; 1.0025x over previous
"""AlphaFold-style gated MSA attention on 8 Trainium2 NeuronCores.

Batch-sharded (128 batches -> 16 per core). Full inputs in, full output out.

Math per batch b (reference):
  q = (q_data @ Wq) * hk^-0.5          [Q, H, 32]
  k = m_data @ Wk ; v = m_data @ Wv    [K, H, 32]
  S[h] = q_h k_h^T + bias[b] + nb[h]   [H, Q, K]
  w = softmax(S, axis=-1)
  wa = w @ v                            [Q, H, 32]
  gate = sigmoid(q_data @ Wg + gb)
  out = (wa * gate).reshape(Q, 256) @ Wo + o_bias

Device-side formulation (per core, per batch):
  All tensors transposed (feature dim on partitions).  S^T[k, q] per head
  from kT/qT projections.  Softmax is unnormalized: the host precomputes
  pw[b, h] = exp(bias[b] + nb[h])^T in bf16 (one DMA per batch) so the
  per-head weights are a single DVE multiply:
      w^T = exp(S^T) * pw[h]
  The V-matmul uses lhsT = [v_h | 2.0] so PSUM row 32/96 accumulates
  2*sum_k w per head (softmax denominators for free).  Head pairs share a
  PSUM bank; evacuated pairs land in wa_all[:, pair, :], from which one
  strided DMA per batch gathers all 8 denominator rows and four per-pair
  DMAs rearrange heads into j-layout waA tiles for the output projection.
  Normalization and gating fuse:
      ga^T = waA * (1 + tanh(x/2 + gb/2)) * recip(2*sum)
  with the per-head recip broadcast across 32 partitions by an indicator
  matmul.  Output projection back to [q, 256] with o_bias added via a
  rank-1 fp8 hi/lo DoubleRow matmul; output stored bf16.
"""

import os
import sys

sys.path.insert(0, "/opt/trn_rl_repo")

import numpy as np
import ml_dtypes
from contextlib import ExitStack

import concourse.bass as bass  # noqa: F401  (engine types)
import concourse.bacc as bacc
import concourse.mybir as mybir
import concourse.tile as tile

BF16 = ml_dtypes.bfloat16
E4M3 = ml_dtypes.float8_e4m3fn

NUM_CORES = 8
B, Q, K, A = 128, 384, 384, 256
H, HD = 8, 32  # heads, head dim
OUT = 256
BPC = B // NUM_CORES  # batches per core


def _env(name, default):
    return os.environ.get(name, default)


def _build_body(ctx, tc, io, bpc):
    nc = tc.nc
    f32, bf = mybir.dt.float32, mybir.dt.bfloat16
    fp8 = mybir.dt.float8e4
    Exp = mybir.ActivationFunctionType.Exp
    Tanh = mybir.ActivationFunctionType.Tanh
    MUL, ADD = mybir.AluOpType.mult, mybir.AluOpType.add
    DR = mybir.MatmulPerfMode.DoubleRow

    const = ctx.enter_context(tc.tile_pool(name="const", bufs=1))
    lp = ctx.enter_context(tc.tile_pool(name="loads", bufs=int(_env("LP_BUFS", "3"))))
    pwp = ctx.enter_context(tc.tile_pool(name="pw", bufs=int(_env("PW_BUFS", "2"))))
    pp = ctx.enter_context(tc.tile_pool(name="proj", bufs=int(_env("PP_BUFS", "4"))))
    wp = ctx.enter_context(tc.tile_pool(name="work", bufs=int(_env("WP_BUFS", "4"))))
    wap = ctx.enter_context(tc.tile_pool(name="wa", bufs=int(_env("WA_BUFS", "2"))))
    gp = ctx.enter_context(tc.tile_pool(name="gating", bufs=int(_env("GP_BUFS", "3"))))
    outp = ctx.enter_context(tc.tile_pool(name="outp", bufs=3))
    # PSUM: 2 x 3 banks (S^T) + 2 x 1 bank (everything else) = 8 banks.
    Sp = ctx.enter_context(tc.tile_pool(name="psum_S", bufs=int(_env("S_BUFS", "2")), space="PSUM"))
    sp = ctx.enter_context(tc.tile_pool(name="psum_sm", bufs=int(_env("SP_BUFS", "2")), space="PSUM"))

    # ---- resident constants ----
    w_sb = {}
    for name in ("wq", "wk", "wv", "wg", "wo"):
        w_sb[name] = const.tile([128, 2, 256], bf, tag=name, name=name)
        nc.sync.dma_start(w_sb[name][:], io[name])
    # o_bias via fp8 hi/lo DoubleRow rank-1: [1,2,128] ones, [1,2,256] bias
    ones2 = const.tile([1, 2, 128], fp8, tag="ones2")
    nc.sync.dma_start(ones2[:], io["ones2"])
    obias2 = const.tile([1, 2, OUT], fp8, tag="obias2")
    nc.sync.dma_start(obias2[:], io["obias2"])
    gbh_sb = const.tile([128, 2], f32, tag="gbh")
    nc.sync.dma_start(gbh_sb[:], io["gbh"])
    if _env("ACT_WARM", "1") == "1":
        # dummy activation right after the first tiny const DMA: pulls the
        # ~1.3us ACT table load off the critical path (exp_and_others holds
        # both Exp and Tanh, so no further loads fire later)
        warm = const.tile([128, 2], f32, tag="warm")
        nc.scalar.activation(warm[:], gbh_sb[:], Exp)
    ind_sb = const.tile([4, 128], bf, tag="ind")
    nc.sync.dma_start(ind_sb[:], io["ind"])
    # duplicated identity for fp8 hi/lo DoubleRow adds of (bias+nb) into S
    idup = const.tile([128, 2, 128], fp8, tag="idup")
    nc.sync.dma_start(idup[:], io["idup"])

    # GPSIMD has no PSUM port: all psum evacuations go ACT or DVE.
    qk_ev = nc.scalar if _env("QK_EV", "dve") == "act" else nc.vector
    vv_ev = nc.scalar if _env("VV_EV", "dve") == "act" else nc.vector
    ob_ev = nc.scalar if _env("OB_EV", "dve") == "act" else nc.vector
    wa_ev = nc.scalar if _env("WA_EV", "dve") == "act" else nc.vector

    def ecopy(eng, dst, src):
        if eng is nc.scalar:
            eng.copy(dst, src)
        else:
            eng.tensor_copy(dst, src)

    from concourse.tile_rust import add_dep_helper as _adh

    wa_war = {}  # buffer-slot -> last strided sums DMA reading that slot
    n_wa_bufs = int(_env("WA_BUFS", "2"))
    states = {}

    def emit_loads(b, ld_pri=None):
        import contextlib
        hp = tc.high_priority() if ld_pri == 0 else contextlib.nullcontext()
        ld = lp.tile([128, 4, Q], bf, tag="ld", name=f"ld_{b}")
        with hp:
            nc.sync.dma_start(ld[:], io["inT"][b])
        cb = pwp.tile([128, H, 3, 2, Q], fp8, tag="cb", name=f"cb_{b}")
        nc.sync.dma_start(cb[:], io["cbT"][b])
        qT = pp.tile([128, 2, Q], bf, tag="qT", name=f"qT_{b}")  # [hc, j, q]
        kT = pp.tile([128, 2, Q], bf, tag="kT", name=f"kT_{b}")  # [hc, j, k]
        gt = pp.tile([128, 2, Q], bf, tag="gt", name=f"gt_{b}")
        # [k, kc, h*33+c | 2.0]; padded to 320 so every head has a 64-wide
        # lhsT window (M=64 writes initialized junk to PSUM rows 32-63,
        # letting the pair evacuation be one full-width copy).
        vv = pp.tile([128, 3, 320], bf, tag="vv", name=f"vv_{b}")
        sums_bf = [
            gp.tile([4, Q], bf, tag=f"sums_bf{i}", name=f"sums_bf{i}_{b}")
            for i in range(2)
        ]
        wa_all = wap.tile([128, 4, Q], bf, tag="wa_all", name=f"wa_all_{b}")
        waA = [
            pp.tile([128, Q], bf, tag=f"waA{j}", name=f"waA{j}_{b}") for j in range(2)
        ]
        states[b] = dict(
            ld=ld, cb=cb, qT=qT, kT=kT, gt=gt, vv=vv,
            sums_bf=sums_bf, wa_all=wa_all, waA=waA, psW=None, psO=None,
            wa_evs=[], ga=[None, None], ob=None, recb=[None, None],
            recb_tc=[None, None],
        )

    def proj_qkg(b, which, j):
        # which: "q" -> qT, "k" -> kT, "g" -> gt (tanh)
        st = states[b]
        ld = st["ld"]
        src_, wname = (ld[:, 0:2, :], "wq") if which == "q" else (
            (ld[:, 2:4, :], "wk") if which == "k" else (ld[:, 0:2, :], "wg")
        )
        ps = sp.tile([128, 512], f32, tag="sm", name=f"ps{which}{j}_{b}")
        for a in range(2):
            nc.tensor.matmul(
                ps[:, :Q],
                w_sb[wname][:, a, 128 * j : 128 * (j + 1)],
                src_[:, a, :],
                start=(a == 0),
                stop=(a == 1),
            )
        if which == "g":
            nc.scalar.activation(
                st["gt"][:, j, :], ps[:, :Q], Tanh, bias=gbh_sb[:, j : j + 1], scale=0.5
            )
        else:
            dst = st["qT"] if which == "q" else st["kT"]
            ecopy(qk_ev, dst[:, j, :], ps[:, :Q])

    def proj_v01(b):
        st = states[b]
        md = st["ld"][:, 2:4, :]
        vv = st["vv"]
        nc.gpsimd.memset(vv[:], 2.0)
        ps = sp.tile([128, 512], f32, tag="sm", name=f"psv01_{b}")
        for kc in range(2):
            for a in range(2):
                nc.tensor.matmul(
                    ps[:, 256 * kc : 256 * (kc + 1)],
                    md[:, a, 128 * kc : 128 * (kc + 1)],
                    w_sb["wv"][:, a, :],
                    start=(a == 0),
                    stop=(a == 1),
                )
        ecopy(
            vv_ev,
            vv[:, 0:2, 0:264].rearrange("p k (h c) -> p k h c", c=33)[:, :, :, 0:32],
            ps[:].rearrange("p (k h c) -> p k h c", k=2, c=32),
        )

    def proj_v2(b):
        st = states[b]
        md = st["ld"][:, 2:4, :]
        ps = sp.tile([128, 512], f32, tag="sm", name=f"psv2_{b}")
        for a in range(2):
            nc.tensor.matmul(
                ps[:, :256],
                md[:, a, 256:384],
                w_sb["wv"][:, a, :],
                start=(a == 0),
                stop=(a == 1),
            )
        ecopy(
            vv_ev,
            st["vv"][:, 2, 0:264].rearrange("p (h c) -> p h c", c=33)[:, :, 0:32],
            ps[:, :256].rearrange("p (h c) -> p h c", c=32),
        )

    PROJ_PIECES = [
        lambda b: proj_qkg(b, "q", 0),
        lambda b: proj_qkg(b, "k", 0),
        lambda b: proj_qkg(b, "q", 1),
        lambda b: proj_qkg(b, "k", 1),
        proj_v01,
        proj_v2,
        lambda b: proj_qkg(b, "g", 0),
        lambda b: proj_qkg(b, "g", 1),
    ]

    def emit_S(b, h):
        st = states[b]
        j, hh = h // 4, h % 4
        psS = Sp.tile([128, 1536], f32, tag="S", name=f"psS{h}_{b}")
        for kc in range(3):
            qk_mm = nc.tensor.matmul(
                psS[:, 512 * kc : 512 * kc + Q],
                st["kT"][32 * hh : 32 * (hh + 1), j, 128 * kc : 128 * (kc + 1)],
                st["qT"][32 * hh : 32 * (hh + 1), j, :],
                start=True,
                stop=False,
                tile_position=(32 * hh, 0),
            )
            # accumulate bias+nb via fp8 hi/lo DoubleRow identity-add
            add_mm = nc.tensor.matmul(
                psS[:, 512 * kc : 512 * kc + Q],
                idup[:],
                st["cb"][:, h, kc, :, :],
                start=False,
                stop=True,
                perf_mode=DR,
            )
            # no data dep links the group members; forbid scheduler reorder
            _adh(add_mm.ins, qk_mm.ins, reason="accumulation group order")
        sview = psS[:].rearrange("p (c x) -> p c x", x=512)[:, :, :Q]
        w = wp.tile([128, 3, Q], bf, tag="w", name=f"w_{h}_{b}",
                    bufs=int(_env("W_BUFS", "9")))
        if _env("EXP_SPLIT", "1") == "2":
            # two ACT ops: frees psS chunks 0-1 for head h+2 sooner
            nc.scalar.activation(w[:, 0:2, :], sview[:, 0:2, :], Exp)
            nc.scalar.activation(w[:, 2, :], sview[:, 2, :], Exp)
        else:
            nc.scalar.activation(w[:], sview, Exp)
        st[f"w{h}"] = w

    def emit_AVpair(b, P):
        st = states[b]
        vv, wa_all, waA = st["vv"], st["wa_all"], st["waA"]
        psW = sp.tile([128, 512], f32, tag="sm", name=f"psW{P}_{b}")
        for p in range(2):
            h = 2 * P + p
            w = st.pop(f"w{h}")
            for kc in range(3):
                nc.tensor.matmul(
                    psW[64 * p : 64 * p + 64, :Q],
                    vv[:, kc, 33 * h : 33 * h + 64],
                    w[:, kc, :],
                    start=(kc == 0),
                    stop=(kc == 2),
                )
        _ev = wa_ev if P < 2 else (
            nc.scalar if _env("WA_EV_LATE", "dve") == "act" else wa_ev
        )
        ev_inst = (
            _ev.copy(wa_all[:, P, :], psW[:, :Q])
            if _ev is nc.scalar
            else _ev.tensor_copy(wa_all[:, P, :], psW[:, :Q])
        )
        st["wa_evs"].append(ev_inst)
        # heads (2P, 2P+1) -> waA[j] rows 64*(P%2) .. +64
        # (contiguous partition ranges both sides: tracker-visible)
        jj = P // 2
        r0 = 64 * (P % 2)
        nc.sync.dma_start(waA[jj][r0 : r0 + 32, :], wa_all[0:32, P, :])
        nc.sync.dma_start(waA[jj][r0 + 32 : r0 + 64, :], wa_all[64:96, P, :])

    def emit_sums(b, half):
        # two DMAs gather the 4 denominator rows for pairs (2*half, 2*half+1):
        # psW row 32 holds head 2P, row 96 head 2P+1.  Contiguous partition
        # ranges on both sides keep the Tile tracker's deps exact.  Row order
        # in sums_bf is (2P, 2P+2, 2P+1, 2P+3); ind compensates.
        st = states[b]
        sb = st["sums_bf"][half]
        wa = st["wa_all"]
        nc.sync.dma_start(sb[0:2, :], wa[32:33, 2 * half : 2 * half + 2, :])
        nc.sync.dma_start(sb[2:4, :], wa[96:97, 2 * half : 2 * half + 2, :])

    def tail_chain(b, half):
        import contextlib
        hp = tc.high_priority(int(_env("CHAIN_PRI", "200"))) if _env(
            "CHAIN_HIPRI", "1"
        ) == "1" else contextlib.nullcontext()
        with hp:
            _tail_chain(b, half)

    def _tail_chain(b, half):
        st = states[b]
        sums_f = gp.tile([4, Q], f32, tag=f"sums_f{half}", name=f"sums_f{half}_{b}")
        nc.vector.tensor_copy(sums_f[:], st["sums_bf"][half][:])
        rec = gp.tile([4, Q], f32, tag=f"rec{half}", name=f"rec{half}_{b}")
        nc.vector.reciprocal_approx_fast(rec[:], sums_f[:])
        recb = gp.tile([4, Q], bf, tag=f"recb{half}", name=f"recb{half}_{b}")
        tci = nc.vector.tensor_copy(recb[:], rec[:])
        st["recb"][half] = recb
        st["recb_tc"][half] = tci

    def tail_gate(b, j):
        st = states[b]
        psR = sp.tile([128, 512], f32, tag="sm", name=f"psR{j}_{b}")
        nc.tensor.matmul(
            psR[:, :Q],
            ind_sb[:],
            st["recb"][j][:],
            start=True,
            stop=True,
        )
        g2 = gp.tile([128, Q], bf, tag="g2", name=f"g2{j}_{b}")
        nc.vector.scalar_tensor_tensor(
            g2[:], st["gt"][:, j, :], 1.0, psR[:, :Q], op0=ADD, op1=MUL
        )
        ga = gp.tile([128, Q], bf, tag="ga", name=f"ga{j}_{b}")
        ga_eng = nc.gpsimd if _env("GA_ENG", "dve") == "gps" else nc.vector
        ga_eng.tensor_tensor(ga[:], st["waA"][j][:], g2[:], op=MUL)
        st["ga"][j] = ga

    def tail_out_mm(b, qc):
        # qc 0..1 share one psum bank (two 256-wide accumulation groups);
        # qc==2 gets its own.  Evacuation happens per 256-wide group when the
        # group stops, keeping bank occupancy short.
        st = states[b]
        if st["ob"] is None:
            st["ob"] = outp.tile([128, 3, OUT], bf, tag="ob", name=f"ob_{b}")
        if qc in (0, 2):
            st["psO"] = sp.tile([128, 512], f32, tag="sm", name=f"psO{qc}_{b}")
        psO = st["psO"]
        off = OUT * (qc % 2)
        for j in range(2):
            nc.tensor.matmul(
                psO[:, off : off + OUT],
                st["ga"][j][:, 128 * qc : 128 * (qc + 1)],
                w_sb["wo"][:, j, :],
                start=(j == 0),
                stop=False,
            )
        # rank-1 accumulate of o_bias (fp8 hi/lo DoubleRow)
        nc.tensor.matmul(
            psO[:, off : off + OUT],
            ones2[:],
            obias2[:],
            start=False,
            stop=True,
            perf_mode=DR,
        )
        ecopy(ob_ev, st["ob"][:, qc, :], psO[:, off : off + OUT])

    def tail_out(b):
        st = states[b]
        nc.sync.dma_start(
            io["out"][b].rearrange("(c p) o -> p c o", p=128), st["ob"][:]
        )

    # ---- slot-woven software pipeline ----
    # slot h of batch b emits: S/exp/mul for head h of b, lagged AV pairs of
    # b (with the first denominator half's recip chain still inside b), the
    # remaining tail of b-1, and proj piece h of b+1.
    emit_loads(0)
    if bpc > 1:
        emit_loads(1)
    for i in range(8):
        PROJ_PIECES[i](0)
    for b in range(bpc):
        for h in range(9):
            if h < 8:
                emit_S(b, h)
            if h == 2:
                emit_AVpair(b, 0)
            elif h == 4:
                emit_AVpair(b, 1)
                emit_sums(b, 0)
            elif h == 6:
                tail_chain(b, 0)
            elif h == 7:
                emit_AVpair(b, 2)
            elif h == 8:
                emit_AVpair(b, 3)
                emit_sums(b, 1)
            if b > 0:
                pb = b - 1
                if h == 0:
                    tail_chain(pb, 1)
                elif h == 1:
                    tail_gate(pb, 0)
                elif h == 2:
                    tail_gate(pb, 1)
                elif h == 3:
                    tail_out_mm(pb, 0)
                elif h == 4:
                    tail_out_mm(pb, 1)
                elif h == 5:
                    tail_out_mm(pb, 2)
                elif h == 6:
                    tail_out(pb)
            if h < 8 and b + 1 < bpc:
                PROJ_PIECES[h](b + 1)
            if h == 2 and b + 2 < bpc:
                emit_loads(b + 2)
        if b - 1 >= 0:
            states.pop(b - 1)
    bl = bpc - 1
    tail_chain(bl, 1)
    tail_gate(bl, 0)
    tail_gate(bl, 1)
    for qc in range(3):
        tail_out_mm(bl, qc)
    tail_out(bl)


def build(bpc=BPC):
    nc = bacc.Bacc(
        "TRN2",
        target_bir_lowering=False,
        debug=False,
        enable_asserts=False,
        num_devices=NUM_CORES,
    )
    f32, bf = mybir.dt.float32, mybir.dt.bfloat16
    fp8 = mybir.dt.float8e4
    io = {
        "inT": nc.dram_tensor("inT", [bpc, 128, 4, Q], bf, kind="ExternalInput").ap(),
        "cbT": nc.dram_tensor(
            "cbT", [bpc, 128, H, 3, 2, Q], fp8, kind="ExternalInput"
        ).ap(),
        "idup": nc.dram_tensor("idup", [128, 2, 128], fp8, kind="ExternalInput").ap(),
        "wq": nc.dram_tensor("wq", [128, 2, 256], bf, kind="ExternalInput").ap(),
        "wk": nc.dram_tensor("wk", [128, 2, 256], bf, kind="ExternalInput").ap(),
        "wv": nc.dram_tensor("wv", [128, 2, 256], bf, kind="ExternalInput").ap(),
        "wg": nc.dram_tensor("wg", [128, 2, 256], bf, kind="ExternalInput").ap(),
        "wo": nc.dram_tensor("wo", [128, 2, 256], bf, kind="ExternalInput").ap(),
        "ones2": nc.dram_tensor("ones2", [1, 2, 128], fp8, kind="ExternalInput").ap(),
        "obias2": nc.dram_tensor("obias2", [1, 2, OUT], fp8, kind="ExternalInput").ap(),
        "gbh": nc.dram_tensor("gbh", [128, 2], f32, kind="ExternalInput").ap(),
        "ind": nc.dram_tensor("ind", [4, 128], bf, kind="ExternalInput").ap(),
        "out": nc.dram_tensor("out", [bpc, Q, OUT], bf, kind="ExternalOutput").ap(),
    }
    with tile.TileContext(nc) as tc:
        with ExitStack() as ctx:
            _build_body(ctx, tc, io, bpc)
    nc.compile()
    return nc


def _prep_inputs(
    q_data,
    m_data,
    bias,
    nonbatched_bias,
    q_weights,
    k_weights,
    v_weights,
    o_weights,
    o_bias,
    gating_w,
    gating_b,
):
    """Host-side preprocessing into the DMA-friendly device layouts."""
    scale = q_weights.shape[-1] ** -0.5

    def featT(x):  # [B, S, A] -> [B, 128, A//128, S]
        b, s, a = x.shape
        t = x.transpose(0, 2, 1).reshape(b, a // 128, 128, s).transpose(0, 2, 1, 3)
        return np.ascontiguousarray(t.astype(BF16))

    qdT = featT(q_data)
    mdT = featT(m_data)
    inT = np.ascontiguousarray(np.concatenate([qdT, mdT], axis=2))

    # cb[b, p, h, kc, t, q] = hi/lo e4m3 split of
    #   bias[b, q, 128*kc+p] + nb[h, q, 128*kc+p]
    bT = bias[:, 0].transpose(0, 2, 1).astype(np.float32)  # [B, K, Q]
    nT = nonbatched_bias.transpose(0, 2, 1).astype(np.float32)  # [H, K, Q]
    comb = bT[:, None] + nT[None]  # [B, H, K, Q]
    hi = comb.astype(E4M3)
    lo = (comb - hi.astype(np.float32)).astype(E4M3)
    cbT = np.stack([hi, lo], axis=-2)  # [B, H, K, 2, Q]
    cbT = np.ascontiguousarray(
        cbT.reshape(B, H, 3, 128, 2, Q).transpose(0, 3, 1, 2, 4, 5)
    )
    idup = np.zeros((128, 2, 128), dtype=E4M3)
    for t in range(2):
        np.fill_diagonal(idup[:, t, :], 1.0)

    def wmat(w, s=1.0):  # [A, H, hd] -> [128, 2, 256]
        m = (w.reshape(A, H * HD) * s).astype(BF16)
        return np.ascontiguousarray(m.reshape(2, 128, 256).transpose(1, 0, 2))

    wq = wmat(q_weights, scale)
    wk = wmat(k_weights)
    wv = wmat(v_weights)
    wg = wmat(gating_w)
    wo = np.ascontiguousarray(
        o_weights.reshape(256, 256).astype(BF16).reshape(2, 128, 256).transpose(1, 0, 2)
    )
    # o_bias rank-1 via fp8 hi/lo DoubleRow: ones2.T @ obias2 = 16*(hi+lo)/16
    # scale bias up x16 before fp8 split, ones row = 1/16 (exact in fp8)
    obf = o_bias.astype(np.float32) * 16.0
    hi = np.clip(obf, -240, 240).astype(E4M3)
    lo = np.clip(obf - hi.astype(np.float32), -240, 240).astype(E4M3)
    obias2 = np.ascontiguousarray(np.stack([hi, lo]).reshape(1, 2, OUT))
    ones2 = np.full((1, 2, 128), 1.0 / 16.0, dtype=E4M3)
    gbh = np.ascontiguousarray(
        (0.5 * gating_b.reshape(H * HD).astype(np.float32)).reshape(2, 128).T
    )
    # indicator for the recip broadcast: sums rows are pair-major = head
    # order within each j-group, so one [4,128] block serves both j's
    ind = np.zeros((4, 128), dtype=BF16)
    for r, hh in enumerate((0, 2, 1, 3)):
        ind[r, 32 * hh : 32 * (hh + 1)] = 1.0
    return dict(
        inT=inT, cbT=cbT, wq=wq, wk=wk, wv=wv, wg=wg, wo=wo,
        ones2=ones2, obias2=obias2, gbh=gbh, ind=ind, idup=idup,
    )


_NC_CACHE = {}


def kernel(**inputs):
    from concourse.bass_utils import run_bass_kernel_spmd

    full = _prep_inputs(**{k: np.asarray(v) for k, v in inputs.items()})
    if BPC not in _NC_CACHE:
        _NC_CACHE[BPC] = build(BPC)
    nc = _NC_CACHE[BPC]

    shared = {
        k: full[k]
        for k in ("wq", "wk", "wv", "wg", "wo", "ones2", "obias2", "gbh", "ind", "idup")
    }
    in_maps = []
    for c in range(NUM_CORES):
        sl = slice(c * BPC, (c + 1) * BPC)
        in_maps.append(dict(inT=full["inT"][sl], cbT=full["cbT"][sl], **shared))

    trace = bool(int(os.environ.get("BASS_KERNEL_TRACE", "0")))
    if trace:
        try:
            from antenv.axon_hooks import get_axon_ntff_profile_hook  # noqa: F401
        except Exception:
            trace = False
    import time

    t0 = time.time()
    res = run_bass_kernel_spmd(
        nc, in_maps, core_ids=list(range(NUM_CORES)), trace=trace
    )
    kernel.last_run_wall_s = time.time() - t0
    if trace and res.exec_time_ns is not None:
        print(f"HW exec time: {res.exec_time_ns} ns")
        kernel.last_exec_time_ns = res.exec_time_ns
    out = np.concatenate([r["out"] for r in res.results], axis=0)
    return out.astype(np.float32)
